# revision 15
# baseline (speedup 1.0000x reference)
"""Trainium2 Bass kernel for a 2-layer GCN (PyG GCNConv semantics).

Strategy (8 NeuronCores, SPMD, full I/O):
  - Host: fold symmetric deg^-1/2 normalization + edge weight into one
    per-edge scalar w~ = dinv[src]*w*dinv[dst]; append self-loop edges
    (w~ = dinv^2). Sort edges by (dst block, src group). Destinations are
    partitioned contiguously across 8 cores (12544 padded nodes each =
    98 blocks of 128). Sources are split into 4 groups of 25088 rows so
    int16 indices work with the fast dma_gather path (4 parallel SWDGE
    queues). The one-hot scatter matrices S (graph-only, shared by both
    layers) are precomputed on the host and streamed from DRAM.
  - Device, per layer (aggregate-first: out = relu((A_hat z) W + b)),
    per dst block:
      for g in 0..3 (parallel SWDGE queues):
        G_g = dma_gather(z_group_g, idx16)      [128e, TBG*128] fp16
              (-1 indices at each group tail are skipped; runtime count
               comes from a value_load of the counts table)
      PSUM aggT[f, n] += G_t.T @ S_t  over tiles (TensorE, fp32 accum)
      out[n, :] = relu(aggT.T @ W + ones.T @ b)  (TensorE f32 + ScalarE)
  - Two launches (one per GCN layer) of the same compiled program; host
    concatenates layer-1 shards, casts to fp16, feeds layer 2.

fp16 data path gives ~2e-4 relative error vs the f32 reference.
"""

import os
from contextlib import ExitStack

import numpy as np

import concourse.bacc as bacc
import concourse.bass as bass
import concourse.mybir as mybir
import concourse.tile as tile
from concourse import bass_utils

P = 128          # partitions / block size / feature dim
D = 128
NCORES = 8
NGROUP = 4                  # src groups (int16 index range)
N_NODES = 100000
NB_PER_CORE = 98            # blocks of 128 dst nodes per core
SHARD = NB_PER_CORE * P     # 12544
N_PAD = SHARD * NCORES      # 100352
GBUFS = 3                   # G pool depth (memset-guarded for -1 skips)

_nc_cache = {}


def build_nc(nb, tbg, nt_rows):
    """Per-core SPMD program: one GCN layer (aggregate + transform)."""
    dt = mybir.dt
    grows = nt_rows // NGROUP
    tb = NGROUP * tbg                 # total tiles per block
    six = tb * 8                      # idx cols (int16): NGROUP * tbg*128/16
    nc = bacc.Bacc(
        "TRN2",
        target_bir_lowering=False,
        debug=False,
        enable_asserts=False,
        num_devices=1,
        num_swdge_queues=4,
    )
    zt = nc.dram_tensor("zt", [nt_rows, D], dt.float16, kind="ExternalInput")
    ixd = nc.dram_tensor("ixd", [nb, P, six], dt.int16, kind="ExternalInput")
    swd = nc.dram_tensor("swd", [nb, P, tb * P], dt.float16, kind="ExternalInput")
    wt = nc.dram_tensor("wt", [D, D], dt.float32, kind="ExternalInput")
    brow = nc.dram_tensor("brow", [1, D], dt.float32, kind="ExternalInput")
    out = nc.dram_tensor("out", [nb * P, D], dt.float32, kind="ExternalOutput")

    with tile.TileContext(nc) as tc, ExitStack() as ctx:
        const = ctx.enter_context(tc.tile_pool(name="const", bufs=1))
        meta = ctx.enter_context(tc.tile_pool(name="meta", bufs=3))
        gpools = [
            ctx.enter_context(tc.tile_pool(name=f"g{g}", bufs=GBUFS))
            for g in range(NGROUP)
        ]
        spool = ctx.enter_context(tc.tile_pool(name="s", bufs=3))
        apool = ctx.enter_context(tc.tile_pool(name="agg", bufs=3))
        opool = ctx.enter_context(tc.tile_pool(name="o", bufs=3))
        ppool = ctx.enter_context(tc.tile_pool(name="ps", bufs=2, space="PSUM"))
        p2pool = ctx.enter_context(tc.tile_pool(name="ps2", bufs=2, space="PSUM"))

        w_t = const.tile([D, D], dt.float32)
        nc.sync.dma_start(out=w_t[:], in_=wt[:])
        b_t = const.tile([1, D], dt.float32)
        nc.sync.dma_start(out=b_t[:], in_=brow[:])
        ones_t = const.tile([1, P], dt.float32)
        nc.vector.memset(ones_t[:], 1.0)

        cap16 = tbg * 8                 # idx cols per group
        for b in range(nb):
            ix = meta.tile([P, six], dt.int16, tag="ix")
            nc.sync.dma_start(out=ix[:], in_=ixd[b])
            s_w = spool.tile([P, tb * P], dt.float16, tag="S")
            nc.sync.dma_start(out=s_w[:], in_=swd[b])

            g_tiles = []
            for g in range(NGROUP):
                g_w = gpools[g].tile([P, tbg * P], dt.float16, tag=f"G{g}")
                nc.gpsimd.dma_gather(
                    out_ap=g_w[:].rearrange("p (j n) -> p j n", n=P),
                    in_ap=zt[g * grows:(g + 1) * grows, :],
                    idxs_ap=ix[:, g * cap16:(g + 1) * cap16],
                    num_idxs=tbg * P,
                    num_idxs_reg=tbg * P,
                    elem_size=P,
                    queue_num=g,
                )
                g_tiles.extend(g_w[:, j * P:(j + 1) * P] for j in range(tbg))

            psum = ppool.tile([P, P], dt.float32, tag="psA")
            for t in range(tb):
                nc.tensor.matmul(
                    out=psum[:],
                    lhsT=g_tiles[t],
                    rhs=s_w[:, t * P:(t + 1) * P],
                    start=(t == 0),
                    stop=(t == tb - 1),
                )

            agg_t = apool.tile([P, P], dt.float32, tag="aggT")
            nc.vector.tensor_copy(out=agg_t[:], in_=psum[:])

            psum2 = p2pool.tile([P, D], dt.float32, tag="psB")
            nc.tensor.matmul(out=psum2[:], lhsT=agg_t[:], rhs=w_t[:],
                             start=True, stop=False)
            nc.tensor.matmul(out=psum2[:], lhsT=ones_t[:], rhs=b_t[:],
                             start=False, stop=True)

            o_t = opool.tile([P, D], dt.float32, tag="o")
            nc.scalar.activation(out=o_t[:], in_=psum2[:],
                                 func=mybir.ActivationFunctionType.Relu)
            nc.sync.dma_start(out=out[b * P:(b + 1) * P, :], in_=o_t[:])

    nc.compile()
    return nc


def preprocess(src, dst, ew, n_nodes, ncores, nb_per_core):
    """Per-core metadata for the dma_gather kernel.

    Returns (ixd, swd, cnt, tbg):
      ixd: [ncores, nb, P, NGROUP*tbg*8] int16 wrapped gather indices,
           replicated across the 8 q7 stripes; -1 padding at group tails
      swd: [ncores, nb, P, NGROUP*tbg*P] fp16 host-built scatter matrices
      cnt: [ncores, 1, nb*NGROUP] int32 real index count per (block, group)
    """
    shard = nb_per_core * P
    n_pad = shard * ncores
    grows = n_pad // NGROUP
    deg = np.bincount(dst, weights=ew.astype(np.float64), minlength=n_nodes) + 1.0
    dinv = (1.0 / np.sqrt(deg)).astype(np.float32)
    loop = np.arange(n_nodes, dtype=np.int64)
    s_all = np.concatenate([src, loop])
    d_all = np.concatenate([dst, loop])
    w_all = np.concatenate([ew.astype(np.float32), np.ones(n_nodes, np.float32)])
    wtil = dinv[s_all] * w_all * dinv[d_all]

    blk = d_all // P
    grp = s_all // grows
    cell = blk * NGROUP + grp
    order = np.lexsort((s_all, cell))
    s_s = s_all[order]
    d_s = d_all[order]
    w_s = wtil[order]
    cell_s = cell[order]

    nblocks = ncores * nb_per_core
    ncells = nblocks * NGROUP
    counts = np.bincount(cell_s, minlength=ncells)
    tbg = max(1, int(-(-counts.max() // P)))
    cap = tbg * P
    starts = np.zeros(ncells, np.int64)
    np.cumsum(counts[:-1], out=starts[1:])
    pos = np.arange(len(d_s)) - starts[cell_s]

    idxp = np.zeros((ncells, cap), np.int16)
    wp = np.zeros((ncells, cap), np.float16)
    slotp = np.zeros((ncells, cap), np.int16)
    flat = cell_s * cap + pos
    idxp.reshape(-1)[flat] = (s_s % grows).astype(np.int16)
    wp.reshape(-1)[flat] = w_s
    slotp.reshape(-1)[flat] = (d_s % P).astype(np.int16)

    # idx: [ncells, cap] -> wrapped [ncells, 16, cap/16] -> 8x stripes
    ixw = idxp.reshape(ncells, cap // 16, 16).transpose(0, 2, 1)
    ixw = np.tile(ixw, (1, 8, 1))
    ixd = ixw.reshape(ncores, nb_per_core, NGROUP, P, cap // 16)
    ixd = np.ascontiguousarray(ixd.transpose(0, 1, 3, 2, 4)).reshape(
        ncores, nb_per_core, P, NGROUP * cap // 16)

    # host-built scatter matrices: S[cell, j, p, n] = w~ * (slot == n)
    onehot = (slotp[:, :, None] == np.arange(P, dtype=np.int16)[None, None, :])
    sw = onehot.astype(np.float16) * wp[:, :, None]       # [ncells, cap, P]
    sw = sw.reshape(ncores, nb_per_core, NGROUP, tbg, P, P)
    swd = np.ascontiguousarray(sw.transpose(0, 1, 4, 2, 3, 5)).reshape(
        ncores, nb_per_core, P, NGROUP * tbg * P)

    return ixd, swd, tbg


def run_layer(nc, z_f16, ixd, swd, W, b, *, trace=False, tmpdir=None):
    ncores = ixd.shape[0]
    in_maps = []
    for c in range(ncores):
        in_maps.append({
            "zt": z_f16,
            "ixd": ixd[c],
            "swd": swd[c],
            "wt": np.ascontiguousarray(W.astype(np.float32)),
            "brow": np.ascontiguousarray(b.astype(np.float32).reshape(1, D)),
        })
    res = bass_utils.run_bass_kernel_spmd(
        nc, in_maps, core_ids=list(range(ncores)), trace=trace, tmpdir=tmpdir,
    )
    out = np.concatenate([res.results[c]["out"] for c in range(ncores)], axis=0)
    return out, res


def _enable_tracing():
    """Install the NTFF profile hook that this image's antenv lacks, and
    neuter the artifact upload (no bucket access here)."""
    import sys
    import types
    try:
        import antenv.axon_hooks  # noqa: F401
        have = True
    except ImportError:
        have = False
    if not have:
        mod = types.ModuleType("antenv.axon_hooks")
        mod._hook = None

        def set_axon_ntff_profile_hook(h):
            mod._hook = h

        def get_axon_ntff_profile_hook():
            return mod._hook

        mod.set_axon_ntff_profile_hook = set_axon_ntff_profile_hook
        mod.get_axon_ntff_profile_hook = get_axon_ntff_profile_hook
        sys.modules["antenv.axon_hooks"] = mod
        from trn_agent_boot.trn_boot import _ntff_profile_via_ctypes
        hook = _ntff_profile_via_ctypes("/opt/axon/libaxon_pjrt.so")
        mod.set_axon_ntff_profile_hook(hook)
    bass_utils.upload_artifacts = lambda tmpdir: f"local:{tmpdir}"


def kernel(x, edge_index, edge_weight, W1, b1, W2, b2):
    x = np.asarray(x, dtype=np.float32)
    edge_index = np.asarray(edge_index)
    edge_weight = np.asarray(edge_weight, dtype=np.float32)
    src = edge_index[0].astype(np.int64)
    dst = edge_index[1].astype(np.int64)

    ixd, swd, tbg = preprocess(src, dst, edge_weight,
                                    N_NODES, NCORES, NB_PER_CORE)

    key = (NB_PER_CORE, tbg, N_PAD)
    if key not in _nc_cache:
        _nc_cache[key] = build_nc(NB_PER_CORE, tbg, N_PAD)
    nc = _nc_cache[key]

    trace = bool(int(os.environ.get("GCN_TRACE", "0")))
    if trace:
        _enable_tracing()

    z1 = np.zeros((N_PAD, D), np.float16)
    z1[:N_NODES] = x.astype(np.float16)
    h1, res1 = run_layer(nc, z1, ixd, swd, W1, b1, trace=trace)

    z2 = h1.astype(np.float16)
    h2, res2 = run_layer(nc, z2, ixd, swd, W2, b2, trace=trace)

    if trace:
        t1 = res1.exec_time_ns or 0
        t2 = res2.exec_time_ns or 0
        print(f"[kernel] layer1 exec: {t1} ns, layer2 exec: {t2} ns, "
              f"total: {t1 + t2} ns")
        kernel.last_exec_ns = t1 + t2
        kernel.last_results = (res1, res2)

    return h2[:N_NODES].astype(np.float32)


# revision 16
# speedup vs baseline: 1.1219x; 1.1219x over previous
"""Trainium2 Bass kernel for a 2-layer GCN (PyG GCNConv semantics).

Strategy (8 NeuronCores, SPMD, full I/O):
  - Host: fold symmetric deg^-1/2 normalization + edge weight into one
    per-edge scalar w~ = dinv[src]*w*dinv[dst]; append self-loop edges
    (w~ = dinv^2). Sort edges by (dst block, src group). Destinations are
    partitioned contiguously across 8 cores (12544 padded nodes each =
    98 blocks of 128). Sources are split into 4 groups of 25088 rows so
    int16 indices work with the fast dma_gather path (4 parallel SWDGE
    queues). The one-hot scatter matrices S (graph-only, shared by both
    layers) are precomputed on the host and streamed from DRAM.
  - Device, per layer (aggregate-first: out = relu((A_hat z) W + b)),
    per dst block:
      for g in 0..3 (parallel SWDGE queues):
        G_g = dma_gather(z_group_g, idx16)      [128e, TBG*128] fp16
              (-1 indices at each group tail are skipped; runtime count
               comes from a value_load of the counts table)
      PSUM aggT[f, n] += G_t.T @ S_t  over tiles (TensorE, fp32 accum)
      out[n, :] = relu(aggT.T @ W + ones.T @ b)  (TensorE f32 + ScalarE)
  - Two launches (one per GCN layer) of the same compiled program; host
    concatenates layer-1 shards, casts to fp16, feeds layer 2.

fp16 data path gives ~2e-4 relative error vs the f32 reference.
"""

import os
from contextlib import ExitStack

import numpy as np

import concourse.bacc as bacc
import concourse.bass as bass
import concourse.mybir as mybir
import concourse.tile as tile
from concourse.tile import add_dep_helper
from concourse import bass_utils

P = 128          # partitions / block size / feature dim
D = 128
NCORES = 8
NGROUP = 4                  # src groups (int16 index range)
N_NODES = 100000
NB_PER_CORE = 98            # blocks of 128 dst nodes per core
SHARD = NB_PER_CORE * P     # 12544
N_PAD = SHARD * NCORES      # 100352
GBUFS = 3                   # G pool depth (memset-guarded for -1 skips)

_nc_cache = {}


def build_nc(nb, tbg, nt_rows):
    """Per-core SPMD program: one GCN layer (aggregate + transform)."""
    dt = mybir.dt
    grows = nt_rows // NGROUP
    tb = NGROUP * tbg                 # total tiles per block
    six = tb * 8                      # idx cols (int16): NGROUP * tbg*128/16
    nc = bacc.Bacc(
        "TRN2",
        target_bir_lowering=False,
        debug=False,
        enable_asserts=False,
        num_devices=1,
        num_swdge_queues=4,
    )
    zt = nc.dram_tensor("zt", [nt_rows, D], dt.float16, kind="ExternalInput")
    ixd = nc.dram_tensor("ixd", [nb, P, six], dt.int16, kind="ExternalInput")
    swd = nc.dram_tensor("swd", [nb, P, tb * P], dt.float16, kind="ExternalInput")
    cnt = nc.dram_tensor("cnt", [1, nb * NGROUP], dt.int32, kind="ExternalInput")
    wt = nc.dram_tensor("wt", [D, D], dt.float32, kind="ExternalInput")
    brow = nc.dram_tensor("brow", [1, D], dt.float32, kind="ExternalInput")
    out = nc.dram_tensor("out", [nb * P, D], dt.float32, kind="ExternalOutput")

    with tile.TileContext(nc) as tc, ExitStack() as ctx:
        const = ctx.enter_context(tc.tile_pool(name="const", bufs=1))
        meta = ctx.enter_context(tc.tile_pool(name="meta", bufs=3))
        gpools = [
            ctx.enter_context(tc.tile_pool(name=f"g{g}", bufs=GBUFS))
            for g in range(NGROUP)
        ]
        spool = ctx.enter_context(tc.tile_pool(name="s", bufs=3))
        apool = ctx.enter_context(tc.tile_pool(name="agg", bufs=3))
        opool = ctx.enter_context(tc.tile_pool(name="o", bufs=3))
        ppool = ctx.enter_context(tc.tile_pool(name="ps", bufs=2, space="PSUM"))
        p2pool = ctx.enter_context(tc.tile_pool(name="ps2", bufs=2, space="PSUM"))

        w_t = const.tile([D, D], dt.float32)
        nc.sync.dma_start(out=w_t[:], in_=wt[:])
        b_t = const.tile([1, D], dt.float32)
        nc.sync.dma_start(out=b_t[:], in_=brow[:])
        ones_t = const.tile([1, P], dt.float32)
        nc.vector.memset(ones_t[:], 1.0)
        cnt_t = const.tile([1, nb * NGROUP], dt.int32)
        nc.sync.dma_start(out=cnt_t[:], in_=cnt[:])

        cap16 = tbg * 8                 # idx cols per group
        prev_gather = None
        for b in range(nb):
            ix = meta.tile([P, six], dt.int16, tag="ix")
            nc.sync.dma_start(out=ix[:], in_=ixd[b])
            s_w = spool.tile([P, tb * P], dt.float16, tag="S")
            nc.sync.dma_start(out=s_w[:], in_=swd[b])

            regs = [nc.gpsimd.alloc_register(f"cnt_{b}_{g}")
                    for g in range(NGROUP)]
            ld = nc.gpsimd.reg_load(
                regs, cnt_t[0:1, b * NGROUP:(b + 1) * NGROUP])
            if prev_gather is not None:
                # keep count registers' live ranges short: don't let the
                # scheduler hoist loads far ahead of their gathers
                add_dep_helper(ld.ins, prev_gather.ins, sync=False,
                               reason="limit cnt register liveness")
            g_tiles = []
            for g in range(NGROUP):
                g_w = gpools[g].tile([P, tbg * P], dt.float16, tag=f"G{g}")
                if b < GBUFS:
                    # first pass over each pool buffer: clear stale SBUF so
                    # rows skipped by -1 indices can't be NaN (w~=0 * NaN)
                    nc.vector.memset(g_w[:], 0.0)
                prev_gather = nc.gpsimd.dma_gather(
                    out_ap=g_w[:].rearrange("p (j n) -> p j n", n=P),
                    in_ap=zt[g * grows:(g + 1) * grows, :],
                    idxs_ap=ix[:, g * cap16:(g + 1) * cap16],
                    num_idxs=tbg * P,
                    num_idxs_reg=regs[g],
                    elem_size=P,
                    queue_num=g,
                )
                g_tiles.extend(g_w[:, j * P:(j + 1) * P] for j in range(tbg))

            psum = ppool.tile([P, P], dt.float32, tag="psA")
            for t in range(tb):
                nc.tensor.matmul(
                    out=psum[:],
                    lhsT=g_tiles[t],
                    rhs=s_w[:, t * P:(t + 1) * P],
                    start=(t == 0),
                    stop=(t == tb - 1),
                )

            agg_t = apool.tile([P, P], dt.float32, tag="aggT")
            nc.vector.tensor_copy(out=agg_t[:], in_=psum[:])

            psum2 = p2pool.tile([P, D], dt.float32, tag="psB")
            nc.tensor.matmul(out=psum2[:], lhsT=agg_t[:], rhs=w_t[:],
                             start=True, stop=False)
            nc.tensor.matmul(out=psum2[:], lhsT=ones_t[:], rhs=b_t[:],
                             start=False, stop=True)

            o_t = opool.tile([P, D], dt.float32, tag="o")
            nc.scalar.activation(out=o_t[:], in_=psum2[:],
                                 func=mybir.ActivationFunctionType.Relu)
            nc.sync.dma_start(out=out[b * P:(b + 1) * P, :], in_=o_t[:])

    nc.compile()
    return nc


def preprocess(src, dst, ew, n_nodes, ncores, nb_per_core):
    """Per-core metadata for the dma_gather kernel.

    Returns (ixd, swd, cnt, tbg):
      ixd: [ncores, nb, P, NGROUP*tbg*8] int16 wrapped gather indices,
           replicated across the 8 q7 stripes; -1 padding at group tails
      swd: [ncores, nb, P, NGROUP*tbg*P] fp16 host-built scatter matrices
      cnt: [ncores, 1, nb*NGROUP] int32 real index count per (block, group)
    """
    shard = nb_per_core * P
    n_pad = shard * ncores
    grows = n_pad // NGROUP
    deg = np.bincount(dst, weights=ew.astype(np.float64), minlength=n_nodes) + 1.0
    dinv = (1.0 / np.sqrt(deg)).astype(np.float32)
    loop = np.arange(n_nodes, dtype=np.int64)
    s_all = np.concatenate([src, loop])
    d_all = np.concatenate([dst, loop])
    w_all = np.concatenate([ew.astype(np.float32), np.ones(n_nodes, np.float32)])
    wtil = dinv[s_all] * w_all * dinv[d_all]

    blk = d_all // P
    grp = s_all // grows
    cell = blk * NGROUP + grp
    order = np.lexsort((s_all, cell))
    s_s = s_all[order]
    d_s = d_all[order]
    w_s = wtil[order]
    cell_s = cell[order]

    nblocks = ncores * nb_per_core
    ncells = nblocks * NGROUP
    counts = np.bincount(cell_s, minlength=ncells)
    tbg = max(1, int(-(-counts.max() // P)))
    cap = tbg * P
    starts = np.zeros(ncells, np.int64)
    np.cumsum(counts[:-1], out=starts[1:])
    pos = np.arange(len(d_s)) - starts[cell_s]

    idxp = np.full((ncells, cap), -1, np.int16)
    wp = np.zeros((ncells, cap), np.float16)
    slotp = np.zeros((ncells, cap), np.int16)
    flat = cell_s * cap + pos
    idxp.reshape(-1)[flat] = (s_s % grows).astype(np.int16)
    wp.reshape(-1)[flat] = w_s
    slotp.reshape(-1)[flat] = (d_s % P).astype(np.int16)
    # >= 1 valid index per cell (empty cells get a dummy idx 0 with w~ = 0)
    empty = counts == 0
    idxp[empty, 0] = 0
    cnt = np.maximum(counts, 1).astype(np.int32)

    # idx: [ncells, cap] -> wrapped [ncells, 16, cap/16] -> 8x stripes
    ixw = idxp.reshape(ncells, cap // 16, 16).transpose(0, 2, 1)
    ixw = np.tile(ixw, (1, 8, 1))
    ixd = ixw.reshape(ncores, nb_per_core, NGROUP, P, cap // 16)
    ixd = np.ascontiguousarray(ixd.transpose(0, 1, 3, 2, 4)).reshape(
        ncores, nb_per_core, P, NGROUP * cap // 16)

    # host-built scatter matrices: S[cell, j, p, n] = w~ * (slot == n)
    onehot = (slotp[:, :, None] == np.arange(P, dtype=np.int16)[None, None, :])
    sw = onehot.astype(np.float16) * wp[:, :, None]       # [ncells, cap, P]
    sw = sw.reshape(ncores, nb_per_core, NGROUP, tbg, P, P)
    swd = np.ascontiguousarray(sw.transpose(0, 1, 4, 2, 3, 5)).reshape(
        ncores, nb_per_core, P, NGROUP * tbg * P)

    cnt = np.ascontiguousarray(cnt.reshape(ncores, 1, nb_per_core * NGROUP))
    return ixd, swd, cnt, tbg


def run_layer(nc, z_f16, ixd, swd, cnt, W, b, *, trace=False, tmpdir=None):
    ncores = ixd.shape[0]
    in_maps = []
    for c in range(ncores):
        in_maps.append({
            "zt": z_f16,
            "ixd": ixd[c],
            "swd": swd[c],
            "cnt": cnt[c],
            "wt": np.ascontiguousarray(W.astype(np.float32)),
            "brow": np.ascontiguousarray(b.astype(np.float32).reshape(1, D)),
        })
    res = bass_utils.run_bass_kernel_spmd(
        nc, in_maps, core_ids=list(range(ncores)), trace=trace, tmpdir=tmpdir,
    )
    out = np.concatenate([res.results[c]["out"] for c in range(ncores)], axis=0)
    return out, res


def _enable_tracing():
    """Install the NTFF profile hook that this image's antenv lacks, and
    neuter the artifact upload (no bucket access here)."""
    import sys
    import types
    try:
        import antenv.axon_hooks  # noqa: F401
        have = True
    except ImportError:
        have = False
    if not have:
        mod = types.ModuleType("antenv.axon_hooks")
        mod._hook = None

        def set_axon_ntff_profile_hook(h):
            mod._hook = h

        def get_axon_ntff_profile_hook():
            return mod._hook

        mod.set_axon_ntff_profile_hook = set_axon_ntff_profile_hook
        mod.get_axon_ntff_profile_hook = get_axon_ntff_profile_hook
        sys.modules["antenv.axon_hooks"] = mod
        from trn_agent_boot.trn_boot import _ntff_profile_via_ctypes
        hook = _ntff_profile_via_ctypes("/opt/axon/libaxon_pjrt.so")
        mod.set_axon_ntff_profile_hook(hook)
    bass_utils.upload_artifacts = lambda tmpdir: f"local:{tmpdir}"


def kernel(x, edge_index, edge_weight, W1, b1, W2, b2):
    x = np.asarray(x, dtype=np.float32)
    edge_index = np.asarray(edge_index)
    edge_weight = np.asarray(edge_weight, dtype=np.float32)
    src = edge_index[0].astype(np.int64)
    dst = edge_index[1].astype(np.int64)

    ixd, swd, cnt, tbg = preprocess(src, dst, edge_weight,
                                    N_NODES, NCORES, NB_PER_CORE)

    key = (NB_PER_CORE, tbg, N_PAD)
    if key not in _nc_cache:
        _nc_cache[key] = build_nc(NB_PER_CORE, tbg, N_PAD)
    nc = _nc_cache[key]

    trace = bool(int(os.environ.get("GCN_TRACE", "0")))
    if trace:
        _enable_tracing()

    z1 = np.zeros((N_PAD, D), np.float16)
    z1[:N_NODES] = x.astype(np.float16)
    h1, res1 = run_layer(nc, z1, ixd, swd, cnt, W1, b1, trace=trace)

    z2 = h1.astype(np.float16)
    h2, res2 = run_layer(nc, z2, ixd, swd, cnt, W2, b2, trace=trace)

    if trace:
        t1 = res1.exec_time_ns or 0
        t2 = res2.exec_time_ns or 0
        print(f"[kernel] layer1 exec: {t1} ns, layer2 exec: {t2} ns, "
              f"total: {t1 + t2} ns")
        kernel.last_exec_ns = t1 + t2
        kernel.last_results = (res1, res2)

    return h2[:N_NODES].astype(np.float32)


# revision 17
# speedup vs baseline: 1.1248x; 1.0026x over previous
"""Trainium2 Bass kernel for a 2-layer GCN (PyG GCNConv semantics).

Strategy (8 NeuronCores, SPMD, full I/O):
  - Host: fold symmetric deg^-1/2 normalization + edge weight into one
    per-edge scalar w~ = dinv[src]*w*dinv[dst]; append self-loop edges
    (w~ = dinv^2). Sort edges by (dst block, src group). Destinations are
    partitioned contiguously across 8 cores (12544 padded nodes each =
    98 blocks of 128). Sources are split into 4 groups of 25088 rows so
    int16 indices work with the fast dma_gather path (4 parallel SWDGE
    queues). The one-hot scatter matrices S (graph-only, shared by both
    layers) are precomputed on the host and streamed from DRAM.
  - Device, per layer (aggregate-first: out = relu((A_hat z) W + b)),
    per dst block:
      for g in 0..3 (parallel SWDGE queues):
        G_g = dma_gather(z_group_g, idx16)      [128e, TBG*128] fp16
              (-1 indices at each group tail are skipped; runtime count
               comes from a value_load of the counts table)
      PSUM aggT[f, n] += G_t.T @ S_t  over tiles (TensorE, fp32 accum)
      out[n, :] = relu(aggT.T @ W + ones.T @ b)  (TensorE f32 + ScalarE)
  - Two launches (one per GCN layer) of the same compiled program; host
    concatenates layer-1 shards, casts to fp16, feeds layer 2.

fp16 data path gives ~2e-4 relative error vs the f32 reference.
"""

import os
from contextlib import ExitStack

import numpy as np

import concourse.bacc as bacc
import concourse.bass as bass
import concourse.mybir as mybir
import concourse.tile as tile
from concourse.tile import add_dep_helper
from concourse import bass_utils

P = 128          # partitions / block size / feature dim
D = 128
NCORES = 8
NGROUP = 4                  # src groups (int16 index range)
N_NODES = 100000
NB_PER_CORE = 98            # blocks of 128 dst nodes per core
SHARD = NB_PER_CORE * P     # 12544
N_PAD = SHARD * NCORES      # 100352
GBUFS = 4                   # G pool depth (memset-guarded for -1 skips)

_nc_cache = {}


def build_nc(nb, tbg, nt_rows):
    """Per-core SPMD program: one GCN layer (aggregate + transform)."""
    dt = mybir.dt
    grows = nt_rows // NGROUP
    tb = NGROUP * tbg                 # total tiles per block
    six = tb * 8                      # idx cols (int16): NGROUP * tbg*128/16
    nc = bacc.Bacc(
        "TRN2",
        target_bir_lowering=False,
        debug=False,
        enable_asserts=False,
        num_devices=1,
        num_swdge_queues=4,
    )
    zt = nc.dram_tensor("zt", [nt_rows, D], dt.float16, kind="ExternalInput")
    ixd = nc.dram_tensor("ixd", [nb, P, six], dt.int16, kind="ExternalInput")
    swd = nc.dram_tensor("swd", [nb, P, tb * P], dt.float16, kind="ExternalInput")
    cnt = nc.dram_tensor("cnt", [1, nb * NGROUP], dt.int32, kind="ExternalInput")
    wt = nc.dram_tensor("wt", [D, D], dt.float32, kind="ExternalInput")
    brow = nc.dram_tensor("brow", [1, D], dt.float32, kind="ExternalInput")
    out = nc.dram_tensor("out", [nb * P, D], dt.float32, kind="ExternalOutput")

    with tile.TileContext(nc) as tc, ExitStack() as ctx:
        const = ctx.enter_context(tc.tile_pool(name="const", bufs=1))
        meta = ctx.enter_context(tc.tile_pool(name="meta", bufs=4))
        gpools = [
            ctx.enter_context(tc.tile_pool(name=f"g{g}", bufs=GBUFS))
            for g in range(NGROUP)
        ]
        spool = ctx.enter_context(tc.tile_pool(name="s", bufs=4))
        apool = ctx.enter_context(tc.tile_pool(name="agg", bufs=3))
        opool = ctx.enter_context(tc.tile_pool(name="o", bufs=3))
        ppool = ctx.enter_context(tc.tile_pool(name="ps", bufs=2, space="PSUM"))
        p2pool = ctx.enter_context(tc.tile_pool(name="ps2", bufs=2, space="PSUM"))

        w_t = const.tile([D, D], dt.float32)
        nc.sync.dma_start(out=w_t[:], in_=wt[:])
        b_t = const.tile([1, D], dt.float32)
        nc.sync.dma_start(out=b_t[:], in_=brow[:])
        ones_t = const.tile([1, P], dt.float32)
        nc.vector.memset(ones_t[:], 1.0)
        cnt_t = const.tile([1, nb * NGROUP], dt.int32)
        nc.sync.dma_start(out=cnt_t[:], in_=cnt[:])

        cap16 = tbg * 8                 # idx cols per group
        prev_gather = None
        for b in range(nb):
            ix = meta.tile([P, six], dt.int16, tag="ix")
            nc.sync.dma_start(out=ix[:], in_=ixd[b])
            s_w = spool.tile([P, tb * P], dt.float16, tag="S")
            nc.scalar.dma_start(out=s_w[:], in_=swd[b])

            regs = [nc.gpsimd.alloc_register(f"cnt_{b}_{g}")
                    for g in range(NGROUP)]
            ld = nc.gpsimd.reg_load(
                regs, cnt_t[0:1, b * NGROUP:(b + 1) * NGROUP])
            if prev_gather is not None:
                # keep count registers' live ranges short: don't let the
                # scheduler hoist loads far ahead of their gathers
                add_dep_helper(ld.ins, prev_gather.ins, sync=False,
                               reason="limit cnt register liveness")
            g_tiles = []
            for g in range(NGROUP):
                g_w = gpools[g].tile([P, tbg * P], dt.float16, tag=f"G{g}")
                if b < GBUFS:
                    # first pass over each pool buffer: clear stale SBUF so
                    # rows skipped by -1 indices can't be NaN (w~=0 * NaN)
                    nc.vector.memset(g_w[:], 0.0)
                prev_gather = nc.gpsimd.dma_gather(
                    out_ap=g_w[:].rearrange("p (j n) -> p j n", n=P),
                    in_ap=zt[g * grows:(g + 1) * grows, :],
                    idxs_ap=ix[:, g * cap16:(g + 1) * cap16],
                    num_idxs=tbg * P,
                    num_idxs_reg=regs[g],
                    elem_size=P,
                    queue_num=g,
                )
                g_tiles.extend(g_w[:, j * P:(j + 1) * P] for j in range(tbg))

            psum = ppool.tile([P, P], dt.float32, tag="psA")
            for t in range(tb):
                nc.tensor.matmul(
                    out=psum[:],
                    lhsT=g_tiles[t],
                    rhs=s_w[:, t * P:(t + 1) * P],
                    start=(t == 0),
                    stop=(t == tb - 1),
                )

            agg_t = apool.tile([P, P], dt.float32, tag="aggT")
            nc.vector.tensor_copy(out=agg_t[:], in_=psum[:])

            psum2 = p2pool.tile([P, D], dt.float32, tag="psB")
            nc.tensor.matmul(out=psum2[:], lhsT=agg_t[:], rhs=w_t[:],
                             start=True, stop=False)
            nc.tensor.matmul(out=psum2[:], lhsT=ones_t[:], rhs=b_t[:],
                             start=False, stop=True)

            o_t = opool.tile([P, D], dt.float32, tag="o")
            nc.scalar.activation(out=o_t[:], in_=psum2[:],
                                 func=mybir.ActivationFunctionType.Relu)
            nc.sync.dma_start(out=out[b * P:(b + 1) * P, :], in_=o_t[:])

    nc.compile()
    return nc


def preprocess(src, dst, ew, n_nodes, ncores, nb_per_core):
    """Per-core metadata for the dma_gather kernel.

    Returns (ixd, swd, cnt, tbg):
      ixd: [ncores, nb, P, NGROUP*tbg*8] int16 wrapped gather indices,
           replicated across the 8 q7 stripes; -1 padding at group tails
      swd: [ncores, nb, P, NGROUP*tbg*P] fp16 host-built scatter matrices
      cnt: [ncores, 1, nb*NGROUP] int32 real index count per (block, group)
    """
    shard = nb_per_core * P
    n_pad = shard * ncores
    grows = n_pad // NGROUP
    deg = np.bincount(dst, weights=ew.astype(np.float64), minlength=n_nodes) + 1.0
    dinv = (1.0 / np.sqrt(deg)).astype(np.float32)
    loop = np.arange(n_nodes, dtype=np.int64)
    s_all = np.concatenate([src, loop])
    d_all = np.concatenate([dst, loop])
    w_all = np.concatenate([ew.astype(np.float32), np.ones(n_nodes, np.float32)])
    wtil = dinv[s_all] * w_all * dinv[d_all]

    blk = d_all // P
    grp = s_all // grows
    cell = blk * NGROUP + grp
    order = np.lexsort((s_all, cell))
    s_s = s_all[order]
    d_s = d_all[order]
    w_s = wtil[order]
    cell_s = cell[order]

    nblocks = ncores * nb_per_core
    ncells = nblocks * NGROUP
    counts = np.bincount(cell_s, minlength=ncells)
    tbg = max(1, int(-(-counts.max() // P)))
    cap = tbg * P
    starts = np.zeros(ncells, np.int64)
    np.cumsum(counts[:-1], out=starts[1:])
    pos = np.arange(len(d_s)) - starts[cell_s]

    idxp = np.full((ncells, cap), -1, np.int16)
    wp = np.zeros((ncells, cap), np.float16)
    slotp = np.zeros((ncells, cap), np.int16)
    flat = cell_s * cap + pos
    idxp.reshape(-1)[flat] = (s_s % grows).astype(np.int16)
    wp.reshape(-1)[flat] = w_s
    slotp.reshape(-1)[flat] = (d_s % P).astype(np.int16)
    # >= 1 valid index per cell (empty cells get a dummy idx 0 with w~ = 0)
    empty = counts == 0
    idxp[empty, 0] = 0
    cnt = np.maximum(counts, 1).astype(np.int32)

    # idx: [ncells, cap] -> wrapped [ncells, 16, cap/16] -> 8x stripes
    ixw = idxp.reshape(ncells, cap // 16, 16).transpose(0, 2, 1)
    ixw = np.tile(ixw, (1, 8, 1))
    ixd = ixw.reshape(ncores, nb_per_core, NGROUP, P, cap // 16)
    ixd = np.ascontiguousarray(ixd.transpose(0, 1, 3, 2, 4)).reshape(
        ncores, nb_per_core, P, NGROUP * cap // 16)

    # host-built scatter matrices: S[cell, j, p, n] = w~ * (slot == n)
    onehot = (slotp[:, :, None] == np.arange(P, dtype=np.int16)[None, None, :])
    sw = onehot.astype(np.float16) * wp[:, :, None]       # [ncells, cap, P]
    sw = sw.reshape(ncores, nb_per_core, NGROUP, tbg, P, P)
    swd = np.ascontiguousarray(sw.transpose(0, 1, 4, 2, 3, 5)).reshape(
        ncores, nb_per_core, P, NGROUP * tbg * P)

    cnt = np.ascontiguousarray(cnt.reshape(ncores, 1, nb_per_core * NGROUP))
    return ixd, swd, cnt, tbg


def run_layer(nc, z_f16, ixd, swd, cnt, W, b, *, trace=False, tmpdir=None):
    ncores = ixd.shape[0]
    in_maps = []
    for c in range(ncores):
        in_maps.append({
            "zt": z_f16,
            "ixd": ixd[c],
            "swd": swd[c],
            "cnt": cnt[c],
            "wt": np.ascontiguousarray(W.astype(np.float32)),
            "brow": np.ascontiguousarray(b.astype(np.float32).reshape(1, D)),
        })
    res = bass_utils.run_bass_kernel_spmd(
        nc, in_maps, core_ids=list(range(ncores)), trace=trace, tmpdir=tmpdir,
    )
    out = np.concatenate([res.results[c]["out"] for c in range(ncores)], axis=0)
    return out, res


def _enable_tracing():
    """Install the NTFF profile hook that this image's antenv lacks, and
    neuter the artifact upload (no bucket access here)."""
    import sys
    import types
    try:
        import antenv.axon_hooks  # noqa: F401
        have = True
    except ImportError:
        have = False
    if not have:
        mod = types.ModuleType("antenv.axon_hooks")
        mod._hook = None

        def set_axon_ntff_profile_hook(h):
            mod._hook = h

        def get_axon_ntff_profile_hook():
            return mod._hook

        mod.set_axon_ntff_profile_hook = set_axon_ntff_profile_hook
        mod.get_axon_ntff_profile_hook = get_axon_ntff_profile_hook
        sys.modules["antenv.axon_hooks"] = mod
        from trn_agent_boot.trn_boot import _ntff_profile_via_ctypes
        hook = _ntff_profile_via_ctypes("/opt/axon/libaxon_pjrt.so")
        mod.set_axon_ntff_profile_hook(hook)
    bass_utils.upload_artifacts = lambda tmpdir: f"local:{tmpdir}"


def kernel(x, edge_index, edge_weight, W1, b1, W2, b2):
    x = np.asarray(x, dtype=np.float32)
    edge_index = np.asarray(edge_index)
    edge_weight = np.asarray(edge_weight, dtype=np.float32)
    src = edge_index[0].astype(np.int64)
    dst = edge_index[1].astype(np.int64)

    ixd, swd, cnt, tbg = preprocess(src, dst, edge_weight,
                                    N_NODES, NCORES, NB_PER_CORE)

    key = (NB_PER_CORE, tbg, N_PAD)
    if key not in _nc_cache:
        _nc_cache[key] = build_nc(NB_PER_CORE, tbg, N_PAD)
    nc = _nc_cache[key]

    trace = bool(int(os.environ.get("GCN_TRACE", "0")))
    if trace:
        _enable_tracing()

    z1 = np.zeros((N_PAD, D), np.float16)
    z1[:N_NODES] = x.astype(np.float16)
    h1, res1 = run_layer(nc, z1, ixd, swd, cnt, W1, b1, trace=trace)

    z2 = h1.astype(np.float16)
    h2, res2 = run_layer(nc, z2, ixd, swd, cnt, W2, b2, trace=trace)

    if trace:
        t1 = res1.exec_time_ns or 0
        t2 = res2.exec_time_ns or 0
        print(f"[kernel] layer1 exec: {t1} ns, layer2 exec: {t2} ns, "
              f"total: {t1 + t2} ns")
        kernel.last_exec_ns = t1 + t2
        kernel.last_results = (res1, res2)

    return h2[:N_NODES].astype(np.float32)


# revision 18
# speedup vs baseline: 1.1357x; 1.0097x over previous
"""Trainium2 Bass kernel for a 2-layer GCN (PyG GCNConv semantics).

Strategy (8 NeuronCores, SPMD, full I/O):
  - Host: fold symmetric deg^-1/2 normalization + edge weight into one
    per-edge scalar w~ = dinv[src]*w*dinv[dst]; append self-loop edges
    (w~ = dinv^2). Sort edges by (dst block, src group). Destinations are
    partitioned contiguously across 8 cores (12544 padded nodes each =
    98 blocks of 128). Sources are split into 4 groups of 25088 rows so
    int16 indices work with the fast dma_gather path (4 parallel SWDGE
    queues). The one-hot scatter matrices S (graph-only, shared by both
    layers) are precomputed on the host and streamed from DRAM.
  - Device, per layer (aggregate-first: out = relu((A_hat z) W + b)),
    per dst block:
      for g in 0..3 (parallel SWDGE queues):
        G_g = dma_gather(z_group_g, idx16)      [128e, TBG*128] fp16
              (-1 indices at each group tail are skipped; runtime count
               comes from a value_load of the counts table)
      PSUM aggT[f, n] += G_t.T @ S_t  over tiles (TensorE, fp32 accum)
      out[n, :] = relu(aggT.T @ W + ones.T @ b)  (TensorE f32 + ScalarE)
  - Two launches (one per GCN layer) of the same compiled program; host
    concatenates layer-1 shards, casts to fp16, feeds layer 2.

fp16 data path gives ~2e-4 relative error vs the f32 reference.
"""

import os
from contextlib import ExitStack

import numpy as np

import concourse.bacc as bacc
import concourse.bass as bass
import concourse.mybir as mybir
import concourse.tile as tile
from concourse.tile import add_dep_helper
from concourse import bass_utils

P = 128          # partitions / block size / feature dim
D = 128
NCORES = 8
NGROUP = 4                  # src groups (int16 index range)
N_NODES = 100000
NB_PER_CORE = 98            # blocks of 128 dst nodes per core
SHARD = NB_PER_CORE * P     # 12544
N_PAD = SHARD * NCORES      # 100352
GBUFS = 4                   # G pool depth (memset-guarded for -1 skips)

_nc_cache = {}


def build_nc(nb, tbg, nt_rows):
    """Per-core SPMD program: one GCN layer (aggregate + transform)."""
    dt = mybir.dt
    grows = nt_rows // NGROUP
    tb = NGROUP * tbg                 # total tiles per block
    six = tb * 8                      # idx cols (int16): NGROUP * tbg*128/16
    nc = bacc.Bacc(
        "TRN2",
        target_bir_lowering=False,
        debug=False,
        enable_asserts=False,
        num_devices=1,
        num_swdge_queues=4,
    )
    zt = nc.dram_tensor("zt", [nt_rows, D], dt.float16, kind="ExternalInput")
    ixd = nc.dram_tensor("ixd", [nb, P, six], dt.int16, kind="ExternalInput")
    swd = nc.dram_tensor("swd", [nb, P, tb * P], dt.float16, kind="ExternalInput")
    cnt = nc.dram_tensor("cnt", [1, nb * NGROUP], dt.int32, kind="ExternalInput")
    wt = nc.dram_tensor("wt", [D, D], dt.float32, kind="ExternalInput")
    brow = nc.dram_tensor("brow", [1, D], dt.float32, kind="ExternalInput")
    out = nc.dram_tensor("out", [nb * P, D], dt.float32, kind="ExternalOutput")

    with tile.TileContext(nc) as tc, ExitStack() as ctx:
        const = ctx.enter_context(tc.tile_pool(name="const", bufs=1))
        meta = ctx.enter_context(tc.tile_pool(name="meta", bufs=4))
        gpools = [
            ctx.enter_context(tc.tile_pool(name=f"g{g}", bufs=GBUFS))
            for g in range(NGROUP)
        ]
        spool = ctx.enter_context(tc.tile_pool(name="s", bufs=4))
        apool = ctx.enter_context(tc.tile_pool(name="agg", bufs=3))
        opool = ctx.enter_context(tc.tile_pool(name="o", bufs=3))
        ppool = ctx.enter_context(tc.tile_pool(name="ps", bufs=2, space="PSUM"))
        p2pool = ctx.enter_context(tc.tile_pool(name="ps2", bufs=2, space="PSUM"))

        w_t = const.tile([D, D], dt.float32)
        nc.sync.dma_start(out=w_t[:], in_=wt[:])
        b_t = const.tile([1, D], dt.float32)
        nc.sync.dma_start(out=b_t[:], in_=brow[:])
        ones_t = const.tile([1, P], dt.float32)
        nc.vector.memset(ones_t[:], 1.0)
        cnt_t = const.tile([1, nb * NGROUP], dt.int32)
        nc.sync.dma_start(out=cnt_t[:], in_=cnt[:])

        cap16 = tbg * 8                 # idx cols per group
        prev_gather = None
        for b in range(nb):
            ix = meta.tile([P, six], dt.int16, tag="ix")
            nc.sync.dma_start(out=ix[:], in_=ixd[b])
            s_w = spool.tile([P, tb * P], dt.float16, tag="S")
            nc.scalar.dma_start(out=s_w[:], in_=swd[b])

            regs = [nc.gpsimd.alloc_register(f"cnt_{b}_{g}")
                    for g in range(NGROUP)]
            ld = nc.gpsimd.reg_load(
                regs, cnt_t[0:1, b * NGROUP:(b + 1) * NGROUP])
            if prev_gather is not None:
                # keep count registers' live ranges short: don't let the
                # scheduler hoist loads far ahead of their gathers
                add_dep_helper(ld.ins, prev_gather.ins, sync=False,
                               reason="limit cnt register liveness")
            g_tiles = []
            for g in range(NGROUP):
                g_w = gpools[g].tile([P, tbg * P], dt.float16, tag=f"G{g}")
                if b < GBUFS:
                    # first pass over each pool buffer: clear stale SBUF so
                    # rows skipped by -1 indices can't be NaN (w~=0 * NaN)
                    nc.vector.memset(g_w[:], 0.0)
                prev_gather = nc.gpsimd.dma_gather(
                    out_ap=g_w[:].rearrange("p (j n) -> p j n", n=P),
                    in_ap=zt[g * grows:(g + 1) * grows, :],
                    idxs_ap=ix[:, g * cap16:(g + 1) * cap16],
                    num_idxs=tbg * P,
                    num_idxs_reg=regs[g],
                    elem_size=P,
                    queue_num=g,
                    single_packet=False,
                )
                g_tiles.extend(g_w[:, j * P:(j + 1) * P] for j in range(tbg))

            psum = ppool.tile([P, P], dt.float32, tag="psA")
            for t in range(tb):
                nc.tensor.matmul(
                    out=psum[:],
                    lhsT=g_tiles[t],
                    rhs=s_w[:, t * P:(t + 1) * P],
                    start=(t == 0),
                    stop=(t == tb - 1),
                )

            agg_t = apool.tile([P, P], dt.float32, tag="aggT")
            nc.vector.tensor_copy(out=agg_t[:], in_=psum[:])

            psum2 = p2pool.tile([P, D], dt.float32, tag="psB")
            nc.tensor.matmul(out=psum2[:], lhsT=agg_t[:], rhs=w_t[:],
                             start=True, stop=False)
            nc.tensor.matmul(out=psum2[:], lhsT=ones_t[:], rhs=b_t[:],
                             start=False, stop=True)

            o_t = opool.tile([P, D], dt.float32, tag="o")
            nc.scalar.activation(out=o_t[:], in_=psum2[:],
                                 func=mybir.ActivationFunctionType.Relu)
            nc.sync.dma_start(out=out[b * P:(b + 1) * P, :], in_=o_t[:])

    nc.compile()
    return nc


def preprocess(src, dst, ew, n_nodes, ncores, nb_per_core):
    """Per-core metadata for the dma_gather kernel.

    Returns (ixd, swd, cnt, tbg):
      ixd: [ncores, nb, P, NGROUP*tbg*8] int16 wrapped gather indices,
           replicated across the 8 q7 stripes; -1 padding at group tails
      swd: [ncores, nb, P, NGROUP*tbg*P] fp16 host-built scatter matrices
      cnt: [ncores, 1, nb*NGROUP] int32 real index count per (block, group)
    """
    shard = nb_per_core * P
    n_pad = shard * ncores
    grows = n_pad // NGROUP
    deg = np.bincount(dst, weights=ew.astype(np.float64), minlength=n_nodes) + 1.0
    dinv = (1.0 / np.sqrt(deg)).astype(np.float32)
    loop = np.arange(n_nodes, dtype=np.int64)
    s_all = np.concatenate([src, loop])
    d_all = np.concatenate([dst, loop])
    w_all = np.concatenate([ew.astype(np.float32), np.ones(n_nodes, np.float32)])
    wtil = dinv[s_all] * w_all * dinv[d_all]

    blk = d_all // P
    grp = s_all // grows
    cell = blk * NGROUP + grp
    order = np.lexsort((s_all, cell))
    s_s = s_all[order]
    d_s = d_all[order]
    w_s = wtil[order]
    cell_s = cell[order]

    nblocks = ncores * nb_per_core
    ncells = nblocks * NGROUP
    counts = np.bincount(cell_s, minlength=ncells)
    tbg = max(1, int(-(-counts.max() // P)))
    cap = tbg * P
    starts = np.zeros(ncells, np.int64)
    np.cumsum(counts[:-1], out=starts[1:])
    pos = np.arange(len(d_s)) - starts[cell_s]

    idxp = np.full((ncells, cap), -1, np.int16)
    wp = np.zeros((ncells, cap), np.float16)
    slotp = np.zeros((ncells, cap), np.int16)
    flat = cell_s * cap + pos
    idxp.reshape(-1)[flat] = (s_s % grows).astype(np.int16)
    wp.reshape(-1)[flat] = w_s
    slotp.reshape(-1)[flat] = (d_s % P).astype(np.int16)
    # >= 1 valid index per cell (empty cells get a dummy idx 0 with w~ = 0)
    empty = counts == 0
    idxp[empty, 0] = 0
    cnt = np.maximum(counts, 1).astype(np.int32)

    # idx: [ncells, cap] -> wrapped [ncells, 16, cap/16] -> 8x stripes
    ixw = idxp.reshape(ncells, cap // 16, 16).transpose(0, 2, 1)
    ixw = np.tile(ixw, (1, 8, 1))
    ixd = ixw.reshape(ncores, nb_per_core, NGROUP, P, cap // 16)
    ixd = np.ascontiguousarray(ixd.transpose(0, 1, 3, 2, 4)).reshape(
        ncores, nb_per_core, P, NGROUP * cap // 16)

    # host-built scatter matrices: S[cell, j, p, n] = w~ * (slot == n)
    onehot = (slotp[:, :, None] == np.arange(P, dtype=np.int16)[None, None, :])
    sw = onehot.astype(np.float16) * wp[:, :, None]       # [ncells, cap, P]
    sw = sw.reshape(ncores, nb_per_core, NGROUP, tbg, P, P)
    swd = np.ascontiguousarray(sw.transpose(0, 1, 4, 2, 3, 5)).reshape(
        ncores, nb_per_core, P, NGROUP * tbg * P)

    cnt = np.ascontiguousarray(cnt.reshape(ncores, 1, nb_per_core * NGROUP))
    return ixd, swd, cnt, tbg


def run_layer(nc, z_f16, ixd, swd, cnt, W, b, *, trace=False, tmpdir=None):
    ncores = ixd.shape[0]
    in_maps = []
    for c in range(ncores):
        in_maps.append({
            "zt": z_f16,
            "ixd": ixd[c],
            "swd": swd[c],
            "cnt": cnt[c],
            "wt": np.ascontiguousarray(W.astype(np.float32)),
            "brow": np.ascontiguousarray(b.astype(np.float32).reshape(1, D)),
        })
    res = bass_utils.run_bass_kernel_spmd(
        nc, in_maps, core_ids=list(range(ncores)), trace=trace, tmpdir=tmpdir,
    )
    out = np.concatenate([res.results[c]["out"] for c in range(ncores)], axis=0)
    return out, res


def _enable_tracing():
    """Install the NTFF profile hook that this image's antenv lacks, and
    neuter the artifact upload (no bucket access here)."""
    import sys
    import types
    try:
        import antenv.axon_hooks  # noqa: F401
        have = True
    except ImportError:
        have = False
    if not have:
        mod = types.ModuleType("antenv.axon_hooks")
        mod._hook = None

        def set_axon_ntff_profile_hook(h):
            mod._hook = h

        def get_axon_ntff_profile_hook():
            return mod._hook

        mod.set_axon_ntff_profile_hook = set_axon_ntff_profile_hook
        mod.get_axon_ntff_profile_hook = get_axon_ntff_profile_hook
        sys.modules["antenv.axon_hooks"] = mod
        from trn_agent_boot.trn_boot import _ntff_profile_via_ctypes
        hook = _ntff_profile_via_ctypes("/opt/axon/libaxon_pjrt.so")
        mod.set_axon_ntff_profile_hook(hook)
    bass_utils.upload_artifacts = lambda tmpdir: f"local:{tmpdir}"


def kernel(x, edge_index, edge_weight, W1, b1, W2, b2):
    x = np.asarray(x, dtype=np.float32)
    edge_index = np.asarray(edge_index)
    edge_weight = np.asarray(edge_weight, dtype=np.float32)
    src = edge_index[0].astype(np.int64)
    dst = edge_index[1].astype(np.int64)

    ixd, swd, cnt, tbg = preprocess(src, dst, edge_weight,
                                    N_NODES, NCORES, NB_PER_CORE)

    key = (NB_PER_CORE, tbg, N_PAD)
    if key not in _nc_cache:
        _nc_cache[key] = build_nc(NB_PER_CORE, tbg, N_PAD)
    nc = _nc_cache[key]

    trace = bool(int(os.environ.get("GCN_TRACE", "0")))
    if trace:
        _enable_tracing()

    z1 = np.zeros((N_PAD, D), np.float16)
    z1[:N_NODES] = x.astype(np.float16)
    h1, res1 = run_layer(nc, z1, ixd, swd, cnt, W1, b1, trace=trace)

    z2 = h1.astype(np.float16)
    h2, res2 = run_layer(nc, z2, ixd, swd, cnt, W2, b2, trace=trace)

    if trace:
        t1 = res1.exec_time_ns or 0
        t2 = res2.exec_time_ns or 0
        print(f"[kernel] layer1 exec: {t1} ns, layer2 exec: {t2} ns, "
              f"total: {t1 + t2} ns")
        kernel.last_exec_ns = t1 + t2
        kernel.last_results = (res1, res2)

    return h2[:N_NODES].astype(np.float32)


# revision 19
# speedup vs baseline: 1.1364x; 1.0006x over previous
"""Trainium2 Bass kernel for a 2-layer GCN (PyG GCNConv semantics).

Strategy (8 NeuronCores, SPMD, full I/O):
  - Host: fold symmetric deg^-1/2 normalization + edge weight into one
    per-edge scalar w~ = dinv[src]*w*dinv[dst]; append self-loop edges
    (w~ = dinv^2). Sort edges by (dst block, src group). Destinations are
    partitioned contiguously across 8 cores (12544 padded nodes each =
    98 blocks of 128). Sources are split into 4 groups of 25088 rows so
    int16 indices work with the fast dma_gather path (4 parallel SWDGE
    queues). The one-hot scatter matrices S (graph-only, shared by both
    layers) are precomputed on the host and streamed from DRAM.
  - Device, per layer (aggregate-first: out = relu((A_hat z) W + b)),
    per dst block:
      for g in 0..3 (parallel SWDGE queues):
        G_g = dma_gather(z_group_g, idx16)      [128e, TBG*128] fp16
              (-1 indices at each group tail are skipped; the runtime
               count comes from a reg_load of the counts table)
      PSUM aggT[f, n] += G_t.T @ S_t  over tiles (TensorE, fp32 accum)
      out[n, :] = relu(aggT.T @ W + ones.T @ b)  (TensorE f32 + ScalarE)
  - Two launches (one per GCN layer) of the same compiled program; host
    concatenates layer-1 shards, casts to fp16, feeds layer 2.

fp16 data path gives ~2e-4 relative error vs the f32 reference.
"""

import os
from contextlib import ExitStack

import numpy as np

import concourse.bacc as bacc
import concourse.bass as bass
import concourse.mybir as mybir
import concourse.tile as tile
from concourse.tile import add_dep_helper
from concourse import bass_utils

P = 128          # partitions / block size / feature dim
D = 128
NCORES = 8
NGROUP = 4                  # src groups (int16 index range)
N_NODES = 100000
NB_PER_CORE = 98            # blocks of 128 dst nodes per core
SHARD = NB_PER_CORE * P     # 12544
N_PAD = SHARD * NCORES      # 100352
GBUFS = 4                   # G pool depth (memset-guarded for -1 skips)

_nc_cache = {}


def build_nc(nb, tbg, nt_rows):
    """Per-core SPMD program: one GCN layer (aggregate + transform)."""
    dt = mybir.dt
    grows = nt_rows // NGROUP
    tb = NGROUP * tbg                 # total tiles per block
    six = tb * 8                      # idx cols (int16): NGROUP * tbg*128/16
    nc = bacc.Bacc(
        "TRN2",
        target_bir_lowering=False,
        debug=False,
        enable_asserts=False,
        num_devices=1,
        num_swdge_queues=4,
    )
    zt = nc.dram_tensor("zt", [nt_rows, D], dt.float16, kind="ExternalInput")
    ixd = nc.dram_tensor("ixd", [nb, P, six], dt.int16, kind="ExternalInput")
    swd = nc.dram_tensor("swd", [nb, P, tb * P], dt.float16, kind="ExternalInput")
    cnt = nc.dram_tensor("cnt", [1, nb * NGROUP], dt.int32, kind="ExternalInput")
    wt = nc.dram_tensor("wt", [D, D], dt.float32, kind="ExternalInput")
    brow = nc.dram_tensor("brow", [1, D], dt.float32, kind="ExternalInput")
    out = nc.dram_tensor("out", [nb * P, D], dt.float32, kind="ExternalOutput")

    with tile.TileContext(nc) as tc, ExitStack() as ctx:
        const = ctx.enter_context(tc.tile_pool(name="const", bufs=1))
        meta = ctx.enter_context(tc.tile_pool(name="meta", bufs=4))
        gpools = [
            ctx.enter_context(tc.tile_pool(name=f"g{g}", bufs=GBUFS))
            for g in range(NGROUP)
        ]
        spool = ctx.enter_context(tc.tile_pool(name="s", bufs=4))
        apool = ctx.enter_context(tc.tile_pool(name="agg", bufs=3))
        opool = ctx.enter_context(tc.tile_pool(name="o", bufs=3))
        ppool = ctx.enter_context(tc.tile_pool(name="ps", bufs=2, space="PSUM"))
        p2pool = ctx.enter_context(tc.tile_pool(name="ps2", bufs=2, space="PSUM"))

        w_t = const.tile([D, D], dt.float32)
        nc.sync.dma_start(out=w_t[:], in_=wt[:])
        b_t = const.tile([1, D], dt.float32)
        nc.sync.dma_start(out=b_t[:], in_=brow[:])
        ones_t = const.tile([1, P], dt.float32)
        nc.vector.memset(ones_t[:], 1.0)
        cnt_t = const.tile([1, nb * NGROUP], dt.int32)
        nc.sync.dma_start(out=cnt_t[:], in_=cnt[:])

        cap16 = tbg * 8                 # idx cols per group
        prev_gather = None
        for b in range(nb):
            ix = meta.tile([P, six], dt.int16, tag="ix")
            nc.sync.dma_start(out=ix[:], in_=ixd[b])
            s_w = spool.tile([P, tb * P], dt.float16, tag="S")
            nc.scalar.dma_start(out=s_w[:], in_=swd[b])

            regs = [nc.gpsimd.alloc_register(f"cnt_{b}_{g}")
                    for g in range(NGROUP)]
            ld = nc.gpsimd.reg_load(
                regs, cnt_t[0:1, b * NGROUP:(b + 1) * NGROUP])
            if prev_gather is not None:
                # keep count registers' live ranges short: don't let the
                # scheduler hoist loads far ahead of their gathers
                add_dep_helper(ld.ins, prev_gather.ins, sync=False,
                               reason="limit cnt register liveness")
            g_tiles = []
            for g in range(NGROUP):
                g_w = gpools[g].tile([P, tbg * P], dt.float16, tag=f"G{g}")
                if b < GBUFS:
                    # first pass over each pool buffer: clear stale SBUF so
                    # rows skipped by -1 indices can't be NaN (w~=0 * NaN)
                    nc.vector.memset(g_w[:], 0.0)
                prev_gather = nc.gpsimd.dma_gather(
                    out_ap=g_w[:].rearrange("p (j n) -> p j n", n=P),
                    in_ap=zt[g * grows:(g + 1) * grows, :],
                    idxs_ap=ix[:, g * cap16:(g + 1) * cap16],
                    num_idxs=tbg * P,
                    num_idxs_reg=regs[g],
                    elem_size=P,
                    queue_num=g,
                    single_packet=False,
                )
                g_tiles.extend(g_w[:, j * P:(j + 1) * P] for j in range(tbg))

            psum = ppool.tile([P, P], dt.float32, tag="psA")
            for t in range(tb):
                nc.tensor.matmul(
                    out=psum[:],
                    lhsT=g_tiles[t],
                    rhs=s_w[:, t * P:(t + 1) * P],
                    start=(t == 0),
                    stop=(t == tb - 1),
                )

            agg_t = apool.tile([P, P], dt.float32, tag="aggT")
            nc.vector.tensor_copy(out=agg_t[:], in_=psum[:])

            psum2 = p2pool.tile([P, D], dt.float32, tag="psB")
            nc.tensor.matmul(out=psum2[:], lhsT=agg_t[:], rhs=w_t[:],
                             start=True, stop=False)
            nc.tensor.matmul(out=psum2[:], lhsT=ones_t[:], rhs=b_t[:],
                             start=False, stop=True)

            o_t = opool.tile([P, D], dt.float32, tag="o")
            nc.scalar.activation(out=o_t[:], in_=psum2[:],
                                 func=mybir.ActivationFunctionType.Relu)
            nc.sync.dma_start(out=out[b * P:(b + 1) * P, :], in_=o_t[:])

    nc.compile()
    return nc


def preprocess(src, dst, ew, n_nodes, ncores, nb_per_core):
    """Per-core metadata for the dma_gather kernel.

    Returns (ixd, swd, cnt, tbg):
      ixd: [ncores, nb, P, NGROUP*tbg*8] int16 wrapped gather indices,
           replicated across the 8 q7 stripes; -1 padding at group tails
      swd: [ncores, nb, P, NGROUP*tbg*P] fp16 host-built scatter matrices
      cnt: [ncores, 1, nb*NGROUP] int32 real index count per (block, group)
    """
    shard = nb_per_core * P
    n_pad = shard * ncores
    grows = n_pad // NGROUP
    deg = np.bincount(dst, weights=ew.astype(np.float64), minlength=n_nodes) + 1.0
    dinv = (1.0 / np.sqrt(deg)).astype(np.float32)
    loop = np.arange(n_nodes, dtype=np.int64)
    s_all = np.concatenate([src, loop])
    d_all = np.concatenate([dst, loop])
    w_all = np.concatenate([ew.astype(np.float32), np.ones(n_nodes, np.float32)])
    wtil = dinv[s_all] * w_all * dinv[d_all]

    blk = d_all // P
    grp = s_all // grows
    cell = blk * NGROUP + grp
    order = np.lexsort((s_all, cell))
    s_s = s_all[order]
    d_s = d_all[order]
    w_s = wtil[order]
    cell_s = cell[order]

    nblocks = ncores * nb_per_core
    ncells = nblocks * NGROUP
    counts = np.bincount(cell_s, minlength=ncells)
    tbg = max(1, int(-(-counts.max() // P)))
    cap = tbg * P
    starts = np.zeros(ncells, np.int64)
    np.cumsum(counts[:-1], out=starts[1:])
    pos = np.arange(len(d_s)) - starts[cell_s]

    idxp = np.full((ncells, cap), -1, np.int16)
    wp = np.zeros((ncells, cap), np.float16)
    slotp = np.zeros((ncells, cap), np.int16)
    flat = cell_s * cap + pos
    idxp.reshape(-1)[flat] = (s_s % grows).astype(np.int16)
    wp.reshape(-1)[flat] = w_s
    slotp.reshape(-1)[flat] = (d_s % P).astype(np.int16)
    # >= 1 valid index per cell (empty cells get a dummy idx 0 with w~ = 0)
    empty = counts == 0
    idxp[empty, 0] = 0
    cnt = np.maximum(counts, 1).astype(np.int32)

    # idx: [ncells, cap] -> wrapped [ncells, 16, cap/16] -> 8x stripes
    ixw = idxp.reshape(ncells, cap // 16, 16).transpose(0, 2, 1)
    ixw = np.tile(ixw, (1, 8, 1))
    ixd = ixw.reshape(ncores, nb_per_core, NGROUP, P, cap // 16)
    ixd = np.ascontiguousarray(ixd.transpose(0, 1, 3, 2, 4)).reshape(
        ncores, nb_per_core, P, NGROUP * cap // 16)

    # host-built scatter matrices: S[cell, j, p, n] = w~ * (slot == n)
    onehot = (slotp[:, :, None] == np.arange(P, dtype=np.int16)[None, None, :])
    sw = onehot.astype(np.float16) * wp[:, :, None]       # [ncells, cap, P]
    sw = sw.reshape(ncores, nb_per_core, NGROUP, tbg, P, P)
    swd = np.ascontiguousarray(sw.transpose(0, 1, 4, 2, 3, 5)).reshape(
        ncores, nb_per_core, P, NGROUP * tbg * P)

    cnt = np.ascontiguousarray(cnt.reshape(ncores, 1, nb_per_core * NGROUP))
    return ixd, swd, cnt, tbg


def run_layer(nc, z_f16, ixd, swd, cnt, W, b, *, trace=False, tmpdir=None):
    ncores = ixd.shape[0]
    in_maps = []
    for c in range(ncores):
        in_maps.append({
            "zt": z_f16,
            "ixd": ixd[c],
            "swd": swd[c],
            "cnt": cnt[c],
            "wt": np.ascontiguousarray(W.astype(np.float32)),
            "brow": np.ascontiguousarray(b.astype(np.float32).reshape(1, D)),
        })
    res = bass_utils.run_bass_kernel_spmd(
        nc, in_maps, core_ids=list(range(ncores)), trace=trace, tmpdir=tmpdir,
    )
    out = np.concatenate([res.results[c]["out"] for c in range(ncores)], axis=0)
    return out, res


def _enable_tracing():
    """Install the NTFF profile hook that this image's antenv lacks, and
    neuter the artifact upload (no bucket access here)."""
    import sys
    import types
    try:
        import antenv.axon_hooks  # noqa: F401
        have = True
    except ImportError:
        have = False
    if not have:
        mod = types.ModuleType("antenv.axon_hooks")
        mod._hook = None

        def set_axon_ntff_profile_hook(h):
            mod._hook = h

        def get_axon_ntff_profile_hook():
            return mod._hook

        mod.set_axon_ntff_profile_hook = set_axon_ntff_profile_hook
        mod.get_axon_ntff_profile_hook = get_axon_ntff_profile_hook
        sys.modules["antenv.axon_hooks"] = mod
        from trn_agent_boot.trn_boot import _ntff_profile_via_ctypes
        hook = _ntff_profile_via_ctypes("/opt/axon/libaxon_pjrt.so")
        mod.set_axon_ntff_profile_hook(hook)
    bass_utils.upload_artifacts = lambda tmpdir: f"local:{tmpdir}"


def kernel(x, edge_index, edge_weight, W1, b1, W2, b2):
    x = np.asarray(x, dtype=np.float32)
    edge_index = np.asarray(edge_index)
    edge_weight = np.asarray(edge_weight, dtype=np.float32)
    src = edge_index[0].astype(np.int64)
    dst = edge_index[1].astype(np.int64)

    ixd, swd, cnt, tbg = preprocess(src, dst, edge_weight,
                                    N_NODES, NCORES, NB_PER_CORE)

    key = (NB_PER_CORE, tbg, N_PAD)
    if key not in _nc_cache:
        _nc_cache[key] = build_nc(NB_PER_CORE, tbg, N_PAD)
    nc = _nc_cache[key]

    trace = bool(int(os.environ.get("GCN_TRACE", "0")))
    if trace:
        _enable_tracing()

    z1 = np.zeros((N_PAD, D), np.float16)
    z1[:N_NODES] = x.astype(np.float16)
    h1, res1 = run_layer(nc, z1, ixd, swd, cnt, W1, b1, trace=trace)

    z2 = h1.astype(np.float16)
    h2, res2 = run_layer(nc, z2, ixd, swd, cnt, W2, b2, trace=trace)

    if trace:
        t1 = res1.exec_time_ns or 0
        t2 = res2.exec_time_ns or 0
        print(f"[kernel] layer1 exec: {t1} ns, layer2 exec: {t2} ns, "
              f"total: {t1 + t2} ns")
        kernel.last_exec_ns = t1 + t2
        kernel.last_results = (res1, res2)

    return h2[:N_NODES].astype(np.float32)


# revision 21
# speedup vs baseline: 1.3564x; 1.1937x over previous
"""Trainium2 Bass kernel for a 2-layer GCN (PyG GCNConv semantics).

Strategy (8 NeuronCores, SPMD, full I/O):
  - Host: fold symmetric deg^-1/2 normalization + edge weight into one
    per-edge scalar w~ = dinv[src]*w*dinv[dst]; append self-loop edges
    (w~ = dinv^2). Sort edges by (dst block, src group). Destinations are
    partitioned contiguously across 8 cores (12544 padded nodes each =
    98 blocks of 128). Sources are split into 4 groups of 25088 rows so
    int16 indices work with the fast dma_gather path (4 parallel SWDGE
    queues). The one-hot scatter matrices S (graph-only, shared by both
    layers) are precomputed on the host and streamed from DRAM.
  - Device, per layer (aggregate-first: out = relu((A_hat z) W + b)),
    per dst block:
      for g in 0..3 (parallel SWDGE queues):
        G_g = dma_gather(z_group_g, idx16)      [128e, TBG*128] fp16
              (-1 indices at each group tail are skipped; the runtime
               count comes from a reg_load of the counts table)
      PSUM aggT[f, n] += G_t.T @ S_t  over tiles (TensorE, fp32 accum)
      out[n, :] = relu(aggT.T @ W + ones.T @ b)  (TensorE f32 + ScalarE)
  - Two launches (one per GCN layer) of the same compiled program; host
    concatenates layer-1 shards, casts to fp16, feeds layer 2.

fp16 data path gives ~2e-4 relative error vs the f32 reference.
"""

import os
from contextlib import ExitStack

import numpy as np

import concourse.bacc as bacc
import concourse.bass as bass
import concourse.mybir as mybir
import concourse.tile as tile
from concourse.tile import add_dep_helper
from concourse import bass_utils

P = 128          # partitions / block size / feature dim
D = 128
NCORES = 8
NGROUP = 4                  # src groups (int16 index range)
N_NODES = 100000
NB_PER_CORE = 98            # blocks of 128 dst nodes per core
SHARD = NB_PER_CORE * P     # 12544
N_PAD = SHARD * NCORES      # 100352
GBUFS = 4                   # G pool depth (memset-guarded for -1 skips)

_nc_cache = {}


def build_nc(nb, tbg, nt_rows):
    """Per-core SPMD program: one GCN layer (aggregate + transform)."""
    dt = mybir.dt
    grows = nt_rows // NGROUP
    tb = NGROUP * tbg                 # total tiles per block
    six = tb * 8                      # idx cols (int16): NGROUP * tbg*128/16
    nc = bacc.Bacc(
        "TRN2",
        target_bir_lowering=False,
        debug=False,
        enable_asserts=False,
        num_devices=1,
        num_swdge_queues=4,
    )
    zt = nc.dram_tensor("zt", [nt_rows, D], dt.float16, kind="ExternalInput")
    ixd = nc.dram_tensor("ixd", [nb, P, six], dt.int16, kind="ExternalInput")
    swd = nc.dram_tensor("swd", [nb, P, (tb + 1) * P], dt.float16,
                         kind="ExternalInput")
    zself = nc.dram_tensor("zself", [nb * P, D], dt.float16,
                           kind="ExternalInput")
    cnt = nc.dram_tensor("cnt", [1, nb * NGROUP], dt.int32, kind="ExternalInput")
    wt = nc.dram_tensor("wt", [D, D], dt.float32, kind="ExternalInput")
    brow = nc.dram_tensor("brow", [1, D], dt.float32, kind="ExternalInput")
    out = nc.dram_tensor("out", [nb * P, D], dt.float32, kind="ExternalOutput")

    with tile.TileContext(nc) as tc, ExitStack() as ctx:
        const = ctx.enter_context(tc.tile_pool(name="const", bufs=1))
        meta = ctx.enter_context(tc.tile_pool(name="meta", bufs=4))
        gpools = [
            ctx.enter_context(tc.tile_pool(name=f"g{g}", bufs=GBUFS))
            for g in range(NGROUP)
        ]
        spool = ctx.enter_context(tc.tile_pool(name="s", bufs=4))
        apool = ctx.enter_context(tc.tile_pool(name="agg", bufs=3))
        opool = ctx.enter_context(tc.tile_pool(name="o", bufs=3))
        ppool = ctx.enter_context(tc.tile_pool(name="ps", bufs=2, space="PSUM"))
        p2pool = ctx.enter_context(tc.tile_pool(name="ps2", bufs=2, space="PSUM"))

        w_t = const.tile([D, D], dt.float32)
        nc.sync.dma_start(out=w_t[:], in_=wt[:])
        b_t = const.tile([1, D], dt.float32)
        nc.sync.dma_start(out=b_t[:], in_=brow[:])
        ones_t = const.tile([1, P], dt.float32)
        nc.vector.memset(ones_t[:], 1.0)
        cnt_t = const.tile([1, nb * NGROUP], dt.int32)
        nc.sync.dma_start(out=cnt_t[:], in_=cnt[:])

        cap16 = tbg * 8                 # idx cols per group
        prev_gather = None
        for b in range(nb):
            ix = meta.tile([P, six], dt.int16, tag="ix")
            nc.sync.dma_start(out=ix[:], in_=ixd[b])
            s_w = spool.tile([P, (tb + 1) * P], dt.float16, tag="S")
            nc.scalar.dma_start(out=s_w[:], in_=swd[b])
            zs = opool.tile([P, D], dt.float16, tag="zs")
            nc.sync.dma_start(out=zs[:], in_=zself[b * P:(b + 1) * P, :])

            regs = [nc.gpsimd.alloc_register(f"cnt_{b}_{g}")
                    for g in range(NGROUP)]
            ld = nc.gpsimd.reg_load(
                regs, cnt_t[0:1, b * NGROUP:(b + 1) * NGROUP])
            if prev_gather is not None:
                # keep count registers' live ranges short: don't let the
                # scheduler hoist loads far ahead of their gathers
                add_dep_helper(ld.ins, prev_gather.ins, sync=False,
                               reason="limit cnt register liveness")
            g_tiles = []
            for g in range(NGROUP):
                g_w = gpools[g].tile([P, tbg * P], dt.float16, tag=f"G{g}")
                if b < GBUFS:
                    # first pass over each pool buffer: clear stale SBUF so
                    # rows skipped by -1 indices can't be NaN (w~=0 * NaN)
                    nc.vector.memset(g_w[:], 0.0)
                prev_gather = nc.gpsimd.dma_gather(
                    out_ap=g_w[:].rearrange("p (j n) -> p j n", n=P),
                    in_ap=zt[g * grows:(g + 1) * grows, :],
                    idxs_ap=ix[:, g * cap16:(g + 1) * cap16],
                    num_idxs=tbg * P,
                    num_idxs_reg=regs[g],
                    elem_size=P,
                    queue_num=g,
                    single_packet=False,
                )
                g_tiles.extend(g_w[:, j * P:(j + 1) * P] for j in range(tbg))

            psum = ppool.tile([P, P], dt.float32, tag="psA")
            for t in range(tb):
                nc.tensor.matmul(
                    out=psum[:],
                    lhsT=g_tiles[t],
                    rhs=s_w[:, t * P:(t + 1) * P],
                    start=(t == 0),
                    stop=False,
                )
            # self-loop contribution: plain sequential load, diagonal S tile
            nc.tensor.matmul(out=psum[:], lhsT=zs[:],
                             rhs=s_w[:, tb * P:(tb + 1) * P],
                             start=False, stop=True)

            agg_t = apool.tile([P, P], dt.float32, tag="aggT")
            nc.vector.tensor_copy(out=agg_t[:], in_=psum[:])

            psum2 = p2pool.tile([P, D], dt.float32, tag="psB")
            nc.tensor.matmul(out=psum2[:], lhsT=agg_t[:], rhs=w_t[:],
                             start=True, stop=False)
            nc.tensor.matmul(out=psum2[:], lhsT=ones_t[:], rhs=b_t[:],
                             start=False, stop=True)

            o_t = opool.tile([P, D], dt.float32, tag="o")
            nc.scalar.activation(out=o_t[:], in_=psum2[:],
                                 func=mybir.ActivationFunctionType.Relu)
            nc.sync.dma_start(out=out[b * P:(b + 1) * P, :], in_=o_t[:])

    nc.compile()
    return nc


def preprocess(src, dst, ew, n_nodes, ncores, nb_per_core):
    """Per-core metadata for the dma_gather kernel.

    Returns (ixd, swd, cnt, tbg):
      ixd: [ncores, nb, P, NGROUP*tbg*8] int16 wrapped gather indices,
           replicated across the 8 q7 stripes; -1 padding at group tails
      swd: [ncores, nb, P, NGROUP*tbg*P] fp16 host-built scatter matrices
      cnt: [ncores, 1, nb*NGROUP] int32 real index count per (block, group)
    """
    shard = nb_per_core * P
    n_pad = shard * ncores
    grows = n_pad // NGROUP
    deg = np.bincount(dst, weights=ew.astype(np.float64), minlength=n_nodes) + 1.0
    dinv = (1.0 / np.sqrt(deg)).astype(np.float32)
    s_all = src
    d_all = dst
    wtil = dinv[s_all] * ew.astype(np.float32) * dinv[d_all]
    wself = np.zeros(n_pad, np.float32)
    wself[:n_nodes] = dinv * dinv            # self-loop weight 1 * dinv^2

    blk = d_all // P
    grp = s_all // grows
    cell = blk * NGROUP + grp
    order = np.lexsort((s_all, cell))
    s_s = s_all[order]
    d_s = d_all[order]
    w_s = wtil[order]
    cell_s = cell[order]

    nblocks = ncores * nb_per_core
    ncells = nblocks * NGROUP
    counts = np.bincount(cell_s, minlength=ncells)
    tbg = max(1, int(-(-counts.max() // P)))
    cap = tbg * P
    starts = np.zeros(ncells, np.int64)
    np.cumsum(counts[:-1], out=starts[1:])
    pos = np.arange(len(d_s)) - starts[cell_s]

    idxp = np.full((ncells, cap), -1, np.int16)
    wp = np.zeros((ncells, cap), np.float16)
    slotp = np.zeros((ncells, cap), np.int16)
    flat = cell_s * cap + pos
    idxp.reshape(-1)[flat] = (s_s % grows).astype(np.int16)
    wp.reshape(-1)[flat] = w_s
    slotp.reshape(-1)[flat] = (d_s % P).astype(np.int16)
    # >= 1 valid index per cell (empty cells get a dummy idx 0 with w~ = 0)
    empty = counts == 0
    idxp[empty, 0] = 0
    cnt = np.maximum(counts, 1).astype(np.int32)

    # idx: [ncells, cap] -> wrapped [ncells, 16, cap/16] -> 8x stripes
    ixw = idxp.reshape(ncells, cap // 16, 16).transpose(0, 2, 1)
    ixw = np.tile(ixw, (1, 8, 1))
    ixd = ixw.reshape(ncores, nb_per_core, NGROUP, P, cap // 16)
    ixd = np.ascontiguousarray(ixd.transpose(0, 1, 3, 2, 4)).reshape(
        ncores, nb_per_core, P, NGROUP * cap // 16)

    # host-built scatter matrices: S[cell, j, p, n] = w~ * (slot == n)
    onehot = (slotp[:, :, None] == np.arange(P, dtype=np.int16)[None, None, :])
    sw = onehot.astype(np.float16) * wp[:, :, None]       # [ncells, cap, P]
    sw = sw.reshape(ncores, nb_per_core, NGROUP, tbg, P, P)
    sw = np.ascontiguousarray(sw.transpose(0, 1, 4, 2, 3, 5)).reshape(
        ncores, nb_per_core, P, NGROUP * tbg * P)
    # trailing diagonal tile: self-loop contribution (no gather needed)
    diag = (np.eye(P, dtype=np.float16)[None, None] *
            wself.astype(np.float16).reshape(ncores, nb_per_core, P)[..., None, :])
    swd = np.concatenate([sw, diag.reshape(ncores, nb_per_core, P, P)], axis=3)

    cnt = np.ascontiguousarray(cnt.reshape(ncores, 1, nb_per_core * NGROUP))
    return ixd, swd, cnt, tbg


def run_layer(nc, z_f16, ixd, swd, cnt, W, b, *, trace=False, tmpdir=None):
    ncores = ixd.shape[0]
    shard = ixd.shape[1] * P
    in_maps = []
    for c in range(ncores):
        in_maps.append({
            "zt": z_f16,
            "zself": z_f16[c * shard:(c + 1) * shard],
            "ixd": ixd[c],
            "swd": swd[c],
            "cnt": cnt[c],
            "wt": np.ascontiguousarray(W.astype(np.float32)),
            "brow": np.ascontiguousarray(b.astype(np.float32).reshape(1, D)),
        })
    res = bass_utils.run_bass_kernel_spmd(
        nc, in_maps, core_ids=list(range(ncores)), trace=trace, tmpdir=tmpdir,
    )
    out = np.concatenate([res.results[c]["out"] for c in range(ncores)], axis=0)
    return out, res


def _enable_tracing():
    """Install the NTFF profile hook that this image's antenv lacks, and
    neuter the artifact upload (no bucket access here)."""
    import sys
    import types
    try:
        import antenv.axon_hooks  # noqa: F401
        have = True
    except ImportError:
        have = False
    if not have:
        mod = types.ModuleType("antenv.axon_hooks")
        mod._hook = None

        def set_axon_ntff_profile_hook(h):
            mod._hook = h

        def get_axon_ntff_profile_hook():
            return mod._hook

        mod.set_axon_ntff_profile_hook = set_axon_ntff_profile_hook
        mod.get_axon_ntff_profile_hook = get_axon_ntff_profile_hook
        sys.modules["antenv.axon_hooks"] = mod
        from trn_agent_boot.trn_boot import _ntff_profile_via_ctypes
        hook = _ntff_profile_via_ctypes("/opt/axon/libaxon_pjrt.so")
        mod.set_axon_ntff_profile_hook(hook)
    bass_utils.upload_artifacts = lambda tmpdir: f"local:{tmpdir}"


def kernel(x, edge_index, edge_weight, W1, b1, W2, b2):
    x = np.asarray(x, dtype=np.float32)
    edge_index = np.asarray(edge_index)
    edge_weight = np.asarray(edge_weight, dtype=np.float32)
    src = edge_index[0].astype(np.int64)
    dst = edge_index[1].astype(np.int64)

    ixd, swd, cnt, tbg = preprocess(src, dst, edge_weight,
                                    N_NODES, NCORES, NB_PER_CORE)

    key = (NB_PER_CORE, tbg, N_PAD)
    if key not in _nc_cache:
        _nc_cache[key] = build_nc(NB_PER_CORE, tbg, N_PAD)
    nc = _nc_cache[key]

    trace = bool(int(os.environ.get("GCN_TRACE", "0")))
    if trace:
        _enable_tracing()

    z1 = np.zeros((N_PAD, D), np.float16)
    z1[:N_NODES] = x.astype(np.float16)
    h1, res1 = run_layer(nc, z1, ixd, swd, cnt, W1, b1, trace=trace)

    z2 = h1.astype(np.float16)
    h2, res2 = run_layer(nc, z2, ixd, swd, cnt, W2, b2, trace=trace)

    if trace:
        t1 = res1.exec_time_ns or 0
        t2 = res2.exec_time_ns or 0
        print(f"[kernel] layer1 exec: {t1} ns, layer2 exec: {t2} ns, "
              f"total: {t1 + t2} ns")
        kernel.last_exec_ns = t1 + t2
        kernel.last_results = (res1, res2)

    return h2[:N_NODES].astype(np.float32)


# revision 23
# speedup vs baseline: 1.3655x; 1.0066x over previous
"""Trainium2 Bass kernel for a 2-layer GCN (PyG GCNConv semantics).

Strategy (8 NeuronCores, SPMD, full I/O):
  - Host: fold symmetric deg^-1/2 normalization + edge weight into one
    per-edge scalar w~ = dinv[src]*w*dinv[dst]. Self-loops skip the gather
    entirely: each core's own contiguous block rows are loaded sequentially
    and folded in via a trailing host-built diagonal S tile (w~ = dinv^2).
    Sort edges by (dst block, src group). Destinations are
    partitioned contiguously across 8 cores (12544 padded nodes each =
    98 blocks of 128). Sources are split into 4 groups of 25088 rows so
    int16 indices work with the fast dma_gather path (4 parallel SWDGE
    queues). The one-hot scatter matrices S (graph-only, shared by both
    layers) are precomputed on the host and streamed from DRAM.
  - Device, per layer (aggregate-first: out = relu((A_hat z) W + b)),
    per dst block:
      for g in 0..3 (parallel SWDGE queues):
        G_g = dma_gather(z_group_g, idx16)      [128e, TBG*128] fp16
              (-1 indices at each group tail are skipped; the runtime
               count comes from a reg_load of the counts table)
      PSUM aggT[f, n] += G_t.T @ S_t  over tiles (TensorE, fp32 accum)
      out[n, :] = relu(aggT.T @ W + ones.T @ b)  (TensorE f32 + ScalarE)
  - Two launches (one per GCN layer) of the same compiled program; host
    concatenates layer-1 shards, casts to fp16, feeds layer 2.

fp16 data path gives ~2e-4 relative error vs the f32 reference.
"""

import os
from contextlib import ExitStack

import numpy as np

import concourse.bacc as bacc
import concourse.bass as bass
import concourse.mybir as mybir
import concourse.tile as tile
from concourse.tile import add_dep_helper
from concourse import bass_utils

P = 128          # partitions / block size / feature dim
D = 128
NCORES = 8
NGROUP = 4                  # src groups (int16 index range)
N_NODES = 100000
NB_PER_CORE = 98            # blocks of 128 dst nodes per core
SHARD = NB_PER_CORE * P     # 12544
N_PAD = SHARD * NCORES      # 100352
GBUFS = 4                   # G pool depth (memset-guarded for -1 skips)

_nc_cache = {}


def build_nc(nb, tbg, nt_rows):
    """Per-core SPMD program: one GCN layer (aggregate + transform)."""
    dt = mybir.dt
    grows = nt_rows // NGROUP
    tb = NGROUP * tbg                 # total tiles per block
    six = tb * 8                      # idx cols (int16): NGROUP * tbg*128/16
    nc = bacc.Bacc(
        "TRN2",
        target_bir_lowering=False,
        debug=False,
        enable_asserts=False,
        num_devices=1,
        num_swdge_queues=4,
    )
    zt = nc.dram_tensor("zt", [nt_rows, D], dt.float16, kind="ExternalInput")
    ixd = nc.dram_tensor("ixd", [nb, P, six], dt.int16, kind="ExternalInput")
    swd = nc.dram_tensor("swd", [nb, P, (tb + 1) * P], dt.float16,
                         kind="ExternalInput")
    zself = nc.dram_tensor("zself", [nb * P, D], dt.float16,
                           kind="ExternalInput")
    cnt = nc.dram_tensor("cnt", [1, nb * NGROUP], dt.int32, kind="ExternalInput")
    wt = nc.dram_tensor("wt", [D, D], dt.float32, kind="ExternalInput")
    brow = nc.dram_tensor("brow", [1, D], dt.float32, kind="ExternalInput")
    out = nc.dram_tensor("out", [nb * P, D], dt.float32, kind="ExternalOutput")

    with tile.TileContext(nc) as tc, ExitStack() as ctx:
        const = ctx.enter_context(tc.tile_pool(name="const", bufs=1))
        meta = ctx.enter_context(tc.tile_pool(name="meta", bufs=4))
        gpools = [
            ctx.enter_context(tc.tile_pool(name=f"g{g}", bufs=GBUFS))
            for g in range(NGROUP)
        ]
        spool = ctx.enter_context(tc.tile_pool(name="s", bufs=4))
        apool = ctx.enter_context(tc.tile_pool(name="agg", bufs=3))
        opool = ctx.enter_context(tc.tile_pool(name="o", bufs=3))
        ppool = ctx.enter_context(tc.tile_pool(name="ps", bufs=2, space="PSUM"))
        p2pool = ctx.enter_context(tc.tile_pool(name="ps2", bufs=2, space="PSUM"))

        w_t = const.tile([D, D], dt.float32)
        nc.sync.dma_start(out=w_t[:], in_=wt[:])
        b_t = const.tile([1, D], dt.float32)
        nc.sync.dma_start(out=b_t[:], in_=brow[:])
        ones_t = const.tile([1, P], dt.float32)
        nc.vector.memset(ones_t[:], 1.0)
        cnt_t = const.tile([1, nb * NGROUP], dt.int32)
        nc.sync.dma_start(out=cnt_t[:], in_=cnt[:])

        cap16 = tbg * 8                 # idx cols per group
        prev_gather = None
        for b in range(nb):
            ix = meta.tile([P, six], dt.int16, tag="ix")
            nc.sync.dma_start(out=ix[:], in_=ixd[b])
            s_w = spool.tile([P, (tb + 1) * P], dt.float16, tag="S")
            nc.scalar.dma_start(out=s_w[:], in_=swd[b])
            zs = opool.tile([P, D], dt.float16, tag="zs")
            nc.sync.dma_start(out=zs[:], in_=zself[b * P:(b + 1) * P, :])

            regs = [nc.gpsimd.alloc_register(f"cnt_{b}_{g}")
                    for g in range(NGROUP)]
            ld = nc.gpsimd.reg_load(
                regs, cnt_t[0:1, b * NGROUP:(b + 1) * NGROUP])
            if prev_gather is not None:
                # keep count registers' live ranges short: don't let the
                # scheduler hoist loads far ahead of their gathers
                add_dep_helper(ld.ins, prev_gather.ins, sync=False,
                               reason="limit cnt register liveness")
            g_tiles = []
            for g in range(NGROUP):
                g_w = gpools[g].tile([P, tbg * P], dt.float16, tag=f"G{g}")
                if b < GBUFS:
                    # first pass over each pool buffer: clear stale SBUF so
                    # rows skipped by -1 indices can't be NaN (w~=0 * NaN)
                    nc.vector.memset(g_w[:], 0.0)
                prev_gather = nc.gpsimd.dma_gather(
                    out_ap=g_w[:].rearrange("p (j n) -> p j n", n=P),
                    in_ap=zt[g * grows:(g + 1) * grows, :],
                    idxs_ap=ix[:, g * cap16:(g + 1) * cap16],
                    num_idxs=tbg * P,
                    num_idxs_reg=regs[g],
                    elem_size=P,
                    queue_num=g,
                    single_packet=False,
                )
                g_tiles.extend(g_w[:, j * P:(j + 1) * P] for j in range(tbg))

            psum = ppool.tile([P, P], dt.float32, tag="psA")
            for t in range(tb):
                nc.tensor.matmul(
                    out=psum[:],
                    lhsT=g_tiles[t],
                    rhs=s_w[:, t * P:(t + 1) * P],
                    start=(t == 0),
                    stop=False,
                )
            # self-loop contribution: plain sequential load, diagonal S tile
            nc.tensor.matmul(out=psum[:], lhsT=zs[:],
                             rhs=s_w[:, tb * P:(tb + 1) * P],
                             start=False, stop=True)

            agg_t = apool.tile([P, P], dt.float32, tag="aggT")
            nc.scalar.activation(out=agg_t[:], in_=psum[:],
                                 func=mybir.ActivationFunctionType.Copy)

            psum2 = p2pool.tile([P, D], dt.float32, tag="psB")
            nc.tensor.matmul(out=psum2[:], lhsT=agg_t[:], rhs=w_t[:],
                             start=True, stop=False)
            nc.tensor.matmul(out=psum2[:], lhsT=ones_t[:], rhs=b_t[:],
                             start=False, stop=True)

            o_t = opool.tile([P, D], dt.float32, tag="o")
            nc.scalar.activation(out=o_t[:], in_=psum2[:],
                                 func=mybir.ActivationFunctionType.Relu)
            nc.sync.dma_start(out=out[b * P:(b + 1) * P, :], in_=o_t[:])

    nc.compile()
    return nc


def preprocess(src, dst, ew, n_nodes, ncores, nb_per_core):
    """Per-core metadata for the dma_gather kernel.

    Returns (ixd, swd, cnt, tbg):
      ixd: [ncores, nb, P, NGROUP*tbg*8] int16 wrapped gather indices,
           replicated across the 8 q7 stripes; -1 padding at group tails
      swd: [ncores, nb, P, NGROUP*tbg*P] fp16 host-built scatter matrices
      cnt: [ncores, 1, nb*NGROUP] int32 real index count per (block, group)
    """
    shard = nb_per_core * P
    n_pad = shard * ncores
    grows = n_pad // NGROUP
    deg = np.bincount(dst, weights=ew.astype(np.float64), minlength=n_nodes) + 1.0
    dinv = (1.0 / np.sqrt(deg)).astype(np.float32)
    s_all = src
    d_all = dst
    wtil = dinv[s_all] * ew.astype(np.float32) * dinv[d_all]
    wself = np.zeros(n_pad, np.float32)
    wself[:n_nodes] = dinv * dinv            # self-loop weight 1 * dinv^2

    blk = d_all // P
    grp = s_all // grows
    cell = blk * NGROUP + grp
    order = np.lexsort((s_all, cell))
    s_s = s_all[order]
    d_s = d_all[order]
    w_s = wtil[order]
    cell_s = cell[order]

    nblocks = ncores * nb_per_core
    ncells = nblocks * NGROUP
    counts = np.bincount(cell_s, minlength=ncells)
    tbg = max(1, int(-(-counts.max() // P)))
    cap = tbg * P
    starts = np.zeros(ncells, np.int64)
    np.cumsum(counts[:-1], out=starts[1:])
    pos = np.arange(len(d_s)) - starts[cell_s]

    idxp = np.full((ncells, cap), -1, np.int16)
    wp = np.zeros((ncells, cap), np.float16)
    slotp = np.zeros((ncells, cap), np.int16)
    flat = cell_s * cap + pos
    idxp.reshape(-1)[flat] = (s_s % grows).astype(np.int16)
    wp.reshape(-1)[flat] = w_s
    slotp.reshape(-1)[flat] = (d_s % P).astype(np.int16)
    # >= 1 valid index per cell (empty cells get a dummy idx 0 with w~ = 0)
    empty = counts == 0
    idxp[empty, 0] = 0
    cnt = np.maximum(counts, 1).astype(np.int32)

    # idx: [ncells, cap] -> wrapped [ncells, 16, cap/16] -> 8x stripes
    ixw = idxp.reshape(ncells, cap // 16, 16).transpose(0, 2, 1)
    ixw = np.tile(ixw, (1, 8, 1))
    ixd = ixw.reshape(ncores, nb_per_core, NGROUP, P, cap // 16)
    ixd = np.ascontiguousarray(ixd.transpose(0, 1, 3, 2, 4)).reshape(
        ncores, nb_per_core, P, NGROUP * cap // 16)

    # host-built scatter matrices: S[cell, j, p, n] = w~ * (slot == n)
    onehot = (slotp[:, :, None] == np.arange(P, dtype=np.int16)[None, None, :])
    sw = onehot.astype(np.float16) * wp[:, :, None]       # [ncells, cap, P]
    sw = sw.reshape(ncores, nb_per_core, NGROUP, tbg, P, P)
    sw = np.ascontiguousarray(sw.transpose(0, 1, 4, 2, 3, 5)).reshape(
        ncores, nb_per_core, P, NGROUP * tbg * P)
    # trailing diagonal tile: self-loop contribution (no gather needed)
    diag = (np.eye(P, dtype=np.float16)[None, None] *
            wself.astype(np.float16).reshape(ncores, nb_per_core, P)[..., None, :])
    swd = np.concatenate([sw, diag.reshape(ncores, nb_per_core, P, P)], axis=3)

    cnt = np.ascontiguousarray(cnt.reshape(ncores, 1, nb_per_core * NGROUP))
    return ixd, swd, cnt, tbg


def run_layer(nc, z_f16, ixd, swd, cnt, W, b, *, trace=False, tmpdir=None):
    ncores = ixd.shape[0]
    shard = ixd.shape[1] * P
    in_maps = []
    for c in range(ncores):
        in_maps.append({
            "zt": z_f16,
            "zself": z_f16[c * shard:(c + 1) * shard],
            "ixd": ixd[c],
            "swd": swd[c],
            "cnt": cnt[c],
            "wt": np.ascontiguousarray(W.astype(np.float32)),
            "brow": np.ascontiguousarray(b.astype(np.float32).reshape(1, D)),
        })
    res = bass_utils.run_bass_kernel_spmd(
        nc, in_maps, core_ids=list(range(ncores)), trace=trace, tmpdir=tmpdir,
    )
    out = np.concatenate([res.results[c]["out"] for c in range(ncores)], axis=0)
    return out, res


def _enable_tracing():
    """Install the NTFF profile hook that this image's antenv lacks, and
    neuter the artifact upload (no bucket access here)."""
    import sys
    import types
    try:
        import antenv.axon_hooks  # noqa: F401
        have = True
    except ImportError:
        have = False
    if not have:
        mod = types.ModuleType("antenv.axon_hooks")
        mod._hook = None

        def set_axon_ntff_profile_hook(h):
            mod._hook = h

        def get_axon_ntff_profile_hook():
            return mod._hook

        mod.set_axon_ntff_profile_hook = set_axon_ntff_profile_hook
        mod.get_axon_ntff_profile_hook = get_axon_ntff_profile_hook
        sys.modules["antenv.axon_hooks"] = mod
        from trn_agent_boot.trn_boot import _ntff_profile_via_ctypes
        hook = _ntff_profile_via_ctypes("/opt/axon/libaxon_pjrt.so")
        mod.set_axon_ntff_profile_hook(hook)
    bass_utils.upload_artifacts = lambda tmpdir: f"local:{tmpdir}"


def kernel(x, edge_index, edge_weight, W1, b1, W2, b2):
    x = np.asarray(x, dtype=np.float32)
    edge_index = np.asarray(edge_index)
    edge_weight = np.asarray(edge_weight, dtype=np.float32)
    src = edge_index[0].astype(np.int64)
    dst = edge_index[1].astype(np.int64)

    ixd, swd, cnt, tbg = preprocess(src, dst, edge_weight,
                                    N_NODES, NCORES, NB_PER_CORE)

    key = (NB_PER_CORE, tbg, N_PAD)
    if key not in _nc_cache:
        _nc_cache[key] = build_nc(NB_PER_CORE, tbg, N_PAD)
    nc = _nc_cache[key]

    trace = bool(int(os.environ.get("GCN_TRACE", "0")))
    if trace:
        _enable_tracing()

    z1 = np.zeros((N_PAD, D), np.float16)
    z1[:N_NODES] = x.astype(np.float16)
    h1, res1 = run_layer(nc, z1, ixd, swd, cnt, W1, b1, trace=trace)

    z2 = h1.astype(np.float16)
    h2, res2 = run_layer(nc, z2, ixd, swd, cnt, W2, b2, trace=trace)

    if trace:
        t1 = res1.exec_time_ns or 0
        t2 = res2.exec_time_ns or 0
        print(f"[kernel] layer1 exec: {t1} ns, layer2 exec: {t2} ns, "
              f"total: {t1 + t2} ns")
        kernel.last_exec_ns = t1 + t2
        kernel.last_results = (res1, res2)

    return h2[:N_NODES].astype(np.float32)
